# revision 40
# baseline (speedup 1.0000x reference)
"""Trainium2 Bass kernel for a heterogeneous GraphConv layer (3 relations).

out = concat([leaky(GC(inst_feat, W_inst, in_*)),     # -> node   (10000)
              leaky(GC(node_feat, W_node, ni_*)),     # -> inst   (100000)
              leaky(GC(svc_feat,  W_svc,  sc_*))])    # -> svc    (20000)

GC(f, W, src, dst) = rsqrt(deg_d) * segsum_dst((rsqrt(deg_s)*f)[src]) @ W + b
(aggregation commutes with the dense @W, so we gather *raw scaled features*
and apply W once per destination tile).

Strategy: destination-sharded across 8 NeuronCores, with host-balanced
dst->(core,tile,slot) assignment (tile edge sums packed to multiples of 128
and rank-matched across cores so the shared program's per-tile block counts
stay near the mean).

DMA-gather cost on TRN2 is per-descriptor: a 512B descriptor costs the same
as 256B (sub-512B transfers pay a 2x bus penalty), so the gather tables are
laid out as PAIR rows [2*128] fp16 = 512B: one descriptor fetches two
feature rows. The per-core table is ordered by first use so the two edges
that introduce a pair of new sources in the same dst tile share one
descriptor ("paired slots"); repeat edges use one half and the other half
rides free (and occasionally serves a second edge whose source lands on the
pair sibling). Per slot there are two one-hot channels dlA/dlB (-1 = unused)
selecting the dst column for the low/high half row.

Device work per 128-slot block: dma_gather 128 pair rows -> [128e, 256f],
DVE tensor_scalar builds one-hot stA[e,d]=(iota==dlA) (and stB for the
pair-slot prefix blocks only), PE accumulates aggT[f,d] += A.T@stA (+B.T@stB)
in PSUM. Per dst tile: PSUM out = u (x) b (rank-1 K=1 fp32 matmul preloading
bias) + aggT.T @ W (fp16), then one ScalarE Lrelu(out * rsqrt_deg_d) and a
grouped fp16 DMA to the output rows.
"""

import os as _os

import numpy as np

SVC_N, INST_N, NODE_N, HID = 20000, 100000, 10000, 128
NCORES = 8
BLK = 128           # slots per one-hot matmul block
# blocks per dma_gather instruction. NOTE: needs single_packet=False — with
# single_packet=True the whole stream coalesces into one DMA packet, which
# caps at 64 descriptors/engine (num_idxs <= 1024); beyond that the exec
# unit faults (NRT_EXEC_UNIT_UNRECOVERABLE).
CHUNK = int(_os.environ.get("GNN_CHUNK", "16"))
GDT = "fp16"
ACT_MODE = "lrelu"  # "lrelu" (HW leaky relu) | "relu" (sim debug)
PAIR = 2 * HID      # table row = pair of feature rows (512B fp16)
# dst tiles batched per epilogue staging buffer / out DMA
OUT_GRP = int(_os.environ.get("GNN_OUT_GRP", "8"))

_cache = {}


def _cdiv(a, b):
    return (a + b - 1) // b


def _rup(a, b):
    return _cdiv(a, b) * b


def _balanced_assign(deg, n_dst, ntiles, rho):
    """Assign dst nodes to (core, tile, slot) packing per-tile SLOT counts
    (estimated as edges*(1-rho), rho = relation merge rate) just under
    multiples of 128 so per-tile block counts carry minimal ceil padding.

    Returns dst_ids[NCORES, ntiles*128] int64 (-1 = pad slot): the global dst
    node stored at each (core, tile, slot).
    """
    order = np.argsort(-deg, kind="stable")
    # snake over cores -> per-core totals equal to within one max-degree
    core_of = np.empty(n_dst, np.int64)
    snake = np.concatenate([np.arange(NCORES), np.arange(NCORES)[::-1]])
    core_of[order] = snake[np.arange(n_dst) % (2 * NCORES)]

    dst_ids = np.full((NCORES, ntiles * 128), -1, np.int64)
    for c in range(NCORES):
        ids = order[core_of[order] == c]  # degree-descending
        w = deg[ids]
        total = int(w.sum())
        # per-bin block quotas covering the estimated slot total
        nblk = max(ntiles, _cdiv(int(total * (1 - rho) * 1.02), 128))
        base, extra = divmod(nblk, ntiles)
        quota = np.full(ntiles, base, np.int64)
        quota[:extra] += 1
        # edge-weight target per bin: rho-deflated, with margin for
        # per-tile merge-rate noise (a single core spilling a tile past
        # k*128 slots bumps that tile to k+1 blocks on ALL cores)
        target = (0.94 * quota * 128 / max(1e-6, 1 - rho)).astype(np.int64)
        binw = np.zeros(ntiles, np.int64)
        binn = np.zeros(ntiles, np.int64)
        bins = [[] for _ in range(ntiles)]
        # greedy: place each dst (deg desc) in the fullest bin it still fits
        # (by weight target and 128-slot cap); else least-filled open bin
        open_bins = list(range(ntiles))
        for i, d in zip(ids, w):
            best, bestw = -1, -1
            for t in open_bins:
                if binw[t] + d <= target[t] and binw[t] > bestw:
                    best, bestw = t, binw[t]
            if best < 0:
                best = min(open_bins, key=lambda x: binw[x] - target[x])
            t = best
            bins[t].append(i)
            binw[t] += d
            binn[t] += 1
            if binn[t] >= 128:
                open_bins.remove(t)
        for t in range(ntiles):
            ids_t = bins[t]
            dst_ids[c, t * 128: t * 128 + len(ids_t)] = ids_t
    return dst_ids


def _pack_core(es, ed, n_src, ntiles):
    """Pair-slot packing for one (core, relation).

    es: edge source node ids; ed: edge dst slot (tile*128 + dst_local).

    The gather table holds one 512B row per USED source: [feat(r), feat(r+1)]
    (staggered duplicate), so descriptor idx r serves edge(s) on table row r
    via channel A and optionally a second edge on row r+1 via channel B.
    Rows are ordered by their tile-usage lists (lexsort) so edges of the same
    tile sit on adjacent rows and merge into shared slots.

    Returns dict with: table_rows (src id per table row), per-tile slot
    arrays (row idx, dlA, dlB), nslot[t], nB[t].
    """
    KEYLEN = 6
    tile_of = (ed >> 7).astype(np.int64)
    dl = (ed & 127).astype(np.int64)

    rows_used = np.unique(es)
    nrows = len(rows_used)
    rid_of = np.full(n_src, -1, np.int64)
    rid_of[rows_used] = np.arange(nrows)

    # tile-usage key per row: first KEYLEN tiles (sorted), padded
    pt = np.unique(np.stack([rid_of[es], tile_of], axis=1), axis=0)
    grp_new = np.r_[True, pt[1:, 0] != pt[:-1, 0]]
    idx = np.arange(len(pt))
    j = idx - np.maximum.accumulate(np.where(grp_new, idx, 0))
    keymat = np.full((nrows, KEYLEN), 32767, np.int64)
    m = j < KEYLEN
    keymat[pt[m, 0], j[m]] = pt[m, 1]
    order = np.lexsort(keymat.T[::-1])
    table_rows = rows_used[order]          # src id at each table position
    pos_of = np.full(n_src, -1, np.int64)
    pos_of[table_rows] = np.arange(nrows)

    r = pos_of[es]
    # per tile: sort edges by table position; pair edges on consecutive
    # positions (runs split on gaps/duplicates, paired (0,1)(2,3)... in-run)
    o = np.lexsort((r, tile_of))
    kt, kr, kdl = tile_of[o], r[o], dl[o]
    brk = np.r_[True, (kt[1:] != kt[:-1]) | (kr[1:] != kr[:-1] + 1)]
    idx = np.arange(len(kt))
    k_in_run = idx - np.maximum.accumulate(np.where(brk, idx, 0))
    run_id = np.cumsum(brk) - 1
    half = k_in_run & 1
    skey = np.stack([run_id, k_in_run >> 1], axis=1)
    uslot, inv = np.unique(skey, axis=0, return_inverse=True)
    ns = len(uslot)
    s_tile = np.zeros(ns, np.int64)
    s_row = np.zeros(ns, np.int64)
    dlA = np.full(ns, -1, np.int64)
    dlB = np.full(ns, -1, np.int64)
    m0 = half == 0
    s_tile[inv[m0]] = kt[m0]
    s_row[inv[m0]] = kr[m0]
    dlA[inv[m0]] = kdl[m0]
    dlB[inv[~m0]] = kdl[~m0]
    has_b = dlB >= 0

    # order slots per tile: B-present first (so stB/matmul-B run only on a
    # prefix of blocks), then by row for gather locality
    so = np.lexsort((s_row, ~has_b, s_tile))
    s_tile, s_row, dlA, dlB, has_b = (
        s_tile[so], s_row[so], dlA[so], dlB[so], has_b[so])

    nslot = np.bincount(s_tile, minlength=ntiles)
    nB = np.bincount(s_tile[has_b], minlength=ntiles)
    tstart = np.r_[0, np.cumsum(nslot)]
    tiles = []
    for t in range(ntiles):
        sl = slice(tstart[t], tstart[t + 1])
        tiles.append((s_row[sl], dlA[sl], dlB[sl]))
    return dict(table_rows=table_rows, tiles=tiles, nslot=nslot, nB=nB)


def _prep_relation(src, dst, n_src, n_dst, feat, rho):
    """Host-side sharding/packing for one relation.

    rho: estimated slot merge rate (pair-served edge fraction) used to pack
    tiles to near-multiple-of-128 slot counts.
    """
    src = np.asarray(src, np.int64)
    dst = np.asarray(dst, np.int64)
    deg_s = np.maximum(np.bincount(src, minlength=n_src), 1).astype(np.float64)
    deg_d_raw = np.bincount(dst, minlength=n_dst)
    deg_d = np.maximum(deg_d_raw, 1).astype(np.float64)
    rs_s = (1.0 / np.sqrt(deg_s)).astype(np.float32)
    rs_d = (1.0 / np.sqrt(deg_d)).astype(np.float32)
    u_d = np.sqrt(deg_d).astype(np.float32)  # ~= 1/rs_d

    feat_s = (np.asarray(feat, np.float32) * rs_s[:, None]).astype(np.float32)

    D = _rup(_cdiv(n_dst, NCORES), 128)  # dst rows per core (padded)
    ntiles = D // 128

    dst_ids = _balanced_assign(deg_d_raw.astype(np.int64), n_dst, ntiles, rho)
    slot_core = np.empty(n_dst, np.int64)
    slot_loc = np.empty(n_dst, np.int64)
    for c in range(NCORES):
        m = dst_ids[c] >= 0
        slot_core[dst_ids[c, m]] = c
        slot_loc[dst_ids[c, m]] = np.nonzero(m)[0]

    e_core = slot_core[dst]
    e_loc = slot_loc[dst]
    cores = []
    for c in range(NCORES):
        m = e_core == c
        pk = _pack_core(src[m], e_loc[m], n_src, ntiles)
        pk["dst_ids"] = dst_ids[c].copy()
        cores.append(pk)

    # rank-match: per core sort its tiles by slot count desc so tile index t
    # has similar (max-over-core) block counts
    for c in range(NCORES):
        pk = cores[c]
        perm = np.argsort(-pk["nslot"], kind="stable")
        pk["tiles"] = [pk["tiles"][t] for t in perm]
        pk["nslot"] = pk["nslot"][perm]
        pk["nB"] = pk["nB"][perm]
        pk["dst_ids"] = pk["dst_ids"].reshape(ntiles, 128)[perm].reshape(-1)

    nslot_all = np.stack([cores[c]["nslot"] for c in range(NCORES)])
    nB_all = np.stack([cores[c]["nB"] for c in range(NCORES)])
    btile = np.maximum(_cdiv(np.max(nslot_all, axis=0), BLK), 1)
    nbtile = np.minimum(_cdiv(np.max(nB_all, axis=0), BLK), btile)
    nrows = max(len(cores[c]["table_rows"]) for c in range(NCORES))

    return dict(cores=cores, btile=btile, nbtile=nbtile, nrows=nrows,
                nblk=int(btile.sum()), ntiles=ntiles, D=D,
                feat_s=feat_s, rs_d=rs_d, u_d=u_d, n_dst=n_dst)


def _build_host_data(inputs):
    rels = [
        # order matters: output rows are [node_out, inst_out, svc_out].
        # rho = measured pair-merge rate per relation on this graph
        _prep_relation(inputs["in_src"], inputs["in_dst"], INST_N, NODE_N,
                       inputs["instance_feat"], rho=0.49),
        _prep_relation(inputs["ni_src"], inputs["ni_dst"], NODE_N, INST_N,
                       inputs["node_feat"], rho=0.30),
        _prep_relation(inputs["sc_src"], inputs["sc_dst"], SVC_N, SVC_N,
                       inputs["svc_feat"], rho=0.41),
    ]
    Ws = [inputs["W_inst"], inputs["W_node"], inputs["W_svc"]]
    bs = [inputs["b_inst"], inputs["b_node"], inputs["b_svc"]]

    nblk_pads = [_rup(r["nblk"], CHUNK) for r in rels]
    nblk_tot = sum(nblk_pads)
    nidx_tot = nblk_tot * BLK
    ntile_tot = sum(r["ntiles"] for r in rels)

    W_cat = np.concatenate([np.asarray(w, np.float32) for w in Ws],
                           axis=1).astype(np.float16)
    b_row = np.concatenate([np.asarray(b, np.float32) for b in bs]
                           )[None, :].astype(np.float16)
    iota = np.tile(np.arange(128, dtype=np.float32), (128, 1)).astype(np.float16)

    in_maps = []
    for c in range(NCORES):
        gidx = np.full(nidx_tot, -1, np.int64)
        dlA = np.full(nidx_tot, -1.0, np.float32)
        dlB = np.full(nidx_tot, -1.0, np.float32)
        tbls = []
        off = 0  # block offset in global stream
        for r in rels:
            pk = r["cores"][c]
            assert r["nrows"] < 32768, "row idx must fit int16"
            # staggered-duplicate pair rows: tbl[i] = [feat(i), feat(i+1)]
            fr = np.zeros((r["nrows"] + 1, HID), np.float16)
            tr = pk["table_rows"]
            fr[:len(tr)] = r["feat_s"][tr].astype(np.float16)
            tbl = np.concatenate([fr[:-1], fr[1:]], axis=1)
            tbls.append(np.ascontiguousarray(tbl))
            for t in range(r["ntiles"]):
                sp, da, db = pk["tiles"][t]
                n = len(sp)
                base = off * BLK
                gidx[base:base + n] = sp
                gidx[base + n: base + int(r["btile"][t]) * BLK] = 0
                dlA[base:base + n] = da
                dlB[base:base + n] = db
                off += int(r["btile"][t])
            # relation-tail pad blocks keep idx -1 (trimmed device-side)
            off = _rup(off, CHUNK)

        idx16 = np.ascontiguousarray(
            gidx.astype(np.int16).reshape(-1, 16).T)
        idx_sb = np.tile(idx16, (8, 1))                          # [128, nidx/16]
        dlA_sb = np.ascontiguousarray(dlA.reshape(nblk_tot, BLK).T)
        dlB_sb = np.ascontiguousarray(dlB.reshape(nblk_tot, BLK).T)

        rs_sb = np.zeros((128, ntile_tot), np.float32)
        u_sb = np.zeros((1, ntile_tot * 128), np.float32)
        t0 = 0
        for r in rels:
            ids = r["cores"][c]["dst_ids"]
            val_rs = np.zeros(r["D"], np.float32)
            val_u = np.zeros(r["D"], np.float32)
            m = ids >= 0
            val_rs[m] = r["rs_d"][ids[m]]
            val_u[m] = r["u_d"][ids[m]]
            rs_sb[:, t0:t0 + r["ntiles"]] = val_rs.reshape(r["ntiles"], 128).T
            u_sb[0, t0 * 128:(t0 + r["ntiles"]) * 128] = val_u
            t0 += r["ntiles"]
        u_sb = u_sb.astype(np.float16)

        in_maps.append({
            "tbl_in": tbls[0],
            "tbl_ni": tbls[1],
            "tbl_sc": tbls[2],
            "idx_sb": np.ascontiguousarray(idx_sb),
            "dlA_sb": dlA_sb,
            "dlB_sb": dlB_sb,
            "rs_sb": rs_sb,
            "u_sb": u_sb,
            "W_cat": np.ascontiguousarray(W_cat),
            "b_row": np.ascontiguousarray(b_row),
            "iota": np.ascontiguousarray(iota),
        })

    meta = dict(
        nblk_tot=nblk_tot, nidx_tot=nidx_tot, ntile_tot=ntile_tot,
        nrowss=[r["nrows"] for r in rels],
        btiles=[r["btile"].tolist() for r in rels],
        nbtiles=[r["nbtile"].tolist() for r in rels],
        ntiles=[r["ntiles"] for r in rels],
        Ds=[r["D"] for r in rels],
        n_dsts=[r["n_dst"] for r in rels],
        dst_ids=[[r["cores"][c]["dst_ids"] for c in range(NCORES)]
                 for r in rels],
    )
    return meta, in_maps


def _build_program(meta):
    import os

    import concourse.bacc as bacc
    import concourse.mybir as mybir
    import concourse.tile as tile

    dbg_max_tiles = int(os.environ.get("GNN_MAX_TILES", "0"))  # 0 = all

    gdt = mybir.dt.float16
    f16 = mybir.dt.float16
    f32 = mybir.dt.float32
    AF = mybir.ActivationFunctionType
    act_fn = AF.Lrelu if ACT_MODE == "lrelu" else AF.Relu

    nblk_tot, nidx_tot, ntile_tot = (meta["nblk_tot"], meta["nidx_tot"],
                                     meta["ntile_tot"])

    nc = bacc.Bacc("TRN2", target_bir_lowering=False, debug=False,
                   enable_asserts=False, num_devices=NCORES)

    tbl_d = [
        nc.dram_tensor("tbl_in", [meta["nrowss"][0], PAIR], gdt,
                       kind="ExternalInput"),
        nc.dram_tensor("tbl_ni", [meta["nrowss"][1], PAIR], gdt,
                       kind="ExternalInput"),
        nc.dram_tensor("tbl_sc", [meta["nrowss"][2], PAIR], gdt,
                       kind="ExternalInput"),
    ]
    idx_d = nc.dram_tensor("idx_sb", [128, nidx_tot // 16], mybir.dt.int16,
                           kind="ExternalInput")
    dlA_d = nc.dram_tensor("dlA_sb", [128, nblk_tot], f32, kind="ExternalInput")
    dlB_d = nc.dram_tensor("dlB_sb", [128, nblk_tot], f32, kind="ExternalInput")
    rs_d = nc.dram_tensor("rs_sb", [128, ntile_tot], f32, kind="ExternalInput")
    u_d = nc.dram_tensor("u_sb", [1, ntile_tot * 128], f16, kind="ExternalInput")
    W_d = nc.dram_tensor("W_cat", [128, 3 * HID], f16, kind="ExternalInput")
    b_d = nc.dram_tensor("b_row", [1, 3 * HID], f16, kind="ExternalInput")
    iota_d = nc.dram_tensor("iota", [128, 128], gdt, kind="ExternalInput")

    # p-major grouped layout: row g*128+p holds OUT_GRP tiles' rows for dst
    # slot p — out DMA descriptors are OUT_GRP*256B contiguous (no sub-512B
    # DMA bus penalty); host assemble untangles
    out_d = [
        nc.dram_tensor(n, [_cdiv(meta["ntiles"][i], OUT_GRP) * 128,
                           OUT_GRP * HID], f16, kind="ExternalOutput")
        for i, n in enumerate(["out_node", "out_inst", "out_svc"])
    ]

    with tile.TileContext(nc) as tc:
        with (
            tc.tile_pool(name="const", bufs=1) as const,
            tc.tile_pool(name="g", bufs=int(os.environ.get("GNN_GBUFS", "6"))) as gpool,
            tc.tile_pool(name="st", bufs=int(os.environ.get("GNN_STBUFS", "48"))) as stpool,
            tc.tile_pool(name="evac", bufs=int(os.environ.get("GNN_EVBUFS", "6"))) as evac,
            tc.tile_pool(name="osb", bufs=int(os.environ.get("GNN_OBUFS", "6"))) as opool,
            tc.tile_pool(name="psA", bufs=5, space="PSUM") as psA,
            tc.tile_pool(name="psO", bufs=3, space="PSUM") as psO,
        ):
            rel_nblks = [sum(meta["btiles"][r]) for r in range(3)]
            rel_base = [0, 0, 0]  # global block base per relation
            for r in range(1, 3):
                rel_base[r] = rel_base[r - 1] + _rup(rel_nblks[r - 1], CHUNK)
            tg_base = [0, meta["ntiles"][0],
                       meta["ntiles"][0] + meta["ntiles"][1]]

            # first gathers depend only on the leading idx slices + dl/iota:
            # load those first so the gather stream starts ASAP (HWDGE is
            # FIFO per issuing engine)
            idx_t = const.tile([128, nidx_tot // 16], mybir.dt.int16)
            heads = []
            for r in range(3):
                h0 = rel_base[r] * BLK // 16
                h1 = min(h0 + 2 * CHUNK * BLK // 16, nidx_tot // 16)
                heads.append((h0, h1))
                nc.sync.dma_start(idx_t[:, h0:h1], idx_d.ap()[:, h0:h1])
            dlA_t = const.tile([128, nblk_tot], f32)
            nc.sync.dma_start(dlA_t[:], dlA_d.ap())
            dlB_t = const.tile([128, nblk_tot], f32)
            nc.sync.dma_start(dlB_t[:], dlB_d.ap())
            iota_t = const.tile([128, 128], gdt)
            nc.sync.dma_start(iota_t[:], iota_d.ap())
            W_t = const.tile([128, 3 * HID], f16)
            nc.sync.dma_start(W_t[:], W_d.ap())
            b_t = const.tile([1, 3 * HID], f16)
            nc.sync.dma_start(b_t[:], b_d.ap())
            u_t = const.tile([1, ntile_tot * 128], f16)
            nc.sync.dma_start(u_t[:], u_d.ap())
            rs_t = const.tile([128, ntile_tot], f32)
            nc.sync.dma_start(rs_t[:], rs_d.ap())
            for r in range(3):
                h1 = heads[r][1]
                end = rel_base[r] * BLK // 16 + \
                    _rup(rel_nblks[r], CHUNK) * BLK // 16
                if h1 < end:
                    nc.sync.dma_start(idx_t[:, h1:end], idx_d.ap()[:, h1:end])

            g_tiles = {}   # global chunk id -> gather tile

            def issue_gather(rel, lci):
                gci = rel_base[rel] // CHUNK + lci
                gt = gpool.tile([128, CHUNK, PAIR], gdt, tag="g")
                nidx = CHUNK * BLK
                # trailing -1 idxs (relation-tail pads) are skipped; trim reg
                real_blocks = max(0, min(CHUNK, rel_nblks[rel] - lci * CHUNK))
                nc.gpsimd.dma_gather(
                    out_ap=gt[:],
                    in_ap=tbl_d[rel].ap(),
                    idxs_ap=idx_t[:, gci * (nidx // 16):(gci + 1) * (nidx // 16)],
                    num_idxs=nidx,
                    num_idxs_reg=max(BLK, real_blocks * BLK),
                    elem_size=PAIR,
                    single_packet=False,
                )
                g_tiles[gci] = gt

            # relation-interleaved tile schedule: spread the epilogue-heavy
            # relation (ni: many tiles, few blocks) evenly across the
            # gather-heavy one (sc) so no engine's work bunches up
            sched = []
            scale = [0.93, 0.95, 1.0]  # in/ni finish early; sc's last blocks
            for r in range(3):         # keep gathers flowing during drain
                btile = meta["btiles"][r]
                o = 0
                for t in range(meta["ntiles"][r]):
                    if dbg_max_tiles and t >= dbg_max_tiles:
                        break
                    # key on end-fraction: the block-heavy relation's last
                    # tile keeps gathers flowing while light tiles' epilogues
                    # drain, shrinking the no-DMA tail
                    sched.append((scale[r] * (o + btile[t])
                                  / max(1, rel_nblks[r]), r, t))
                    o += btile[t]
            sched.sort()

            rel_blk = [0, 0, 0]       # relation-local block cursor
            osb_state = [None, None, None]

            for _, rel, t in sched:
                ntiles = meta["ntiles"][rel]
                nb = meta["btiles"][rel][t]
                nbB = meta["nbtiles"][rel][t]
                agg = psA.tile([128, 128], f32, tag="agg")
                for b in range(nb):
                    lb = rel_blk[rel]
                    lci, cj = divmod(lb, CHUNK)
                    if cj == 0:
                        issue_gather(rel, lci)
                    gci = rel_base[rel] // CHUNK + lci
                    blk = rel_base[rel] + lb  # global dl column
                    has_b = b < nbB
                    stA = stpool.tile([128, 128], gdt, tag="stA")
                    nc.vector.tensor_scalar(
                        stA[:], iota_t[:], dlA_t[:, blk:blk + 1], None,
                        mybir.AluOpType.is_equal)
                    last = b == nb - 1
                    nc.tensor.matmul(
                        agg[:], g_tiles[gci][:, cj, 0:HID], stA[:],
                        start=(b == 0), stop=(last and not has_b))
                    if has_b:
                        stB = stpool.tile([128, 128], gdt, tag="stB")
                        nc.vector.tensor_scalar(
                            stB[:], iota_t[:], dlB_t[:, blk:blk + 1], None,
                            mybir.AluOpType.is_equal)
                        nc.tensor.matmul(
                            agg[:], g_tiles[gci][:, cj, HID:PAIR], stB[:],
                            start=False, stop=last)
                    rel_blk[rel] += 1
                tglob = tg_base[rel] + t
                aggsb = evac.tile([128, 128], f16, tag="evac")
                nc.scalar.copy(aggsb[:], agg[:])
                po = psO.tile([128, 128], f32, tag="po")
                nc.tensor.matmul(
                    po[:], u_t[:, tglob * 128:(tglob + 1) * 128],
                    b_t[:, rel * HID:(rel + 1) * HID],
                    start=True, stop=False, skip_group_check=True)
                nc.tensor.matmul(
                    po[:], aggsb[:], W_t[:, rel * HID:(rel + 1) * HID],
                    start=False, stop=True, skip_group_check=True)
                oj = t % OUT_GRP
                if oj == 0:
                    osb_state[rel] = (
                        opool.tile([128, OUT_GRP, 128], f16, tag="osb",
                                   name="osb"), t)
                osb, osb_t0 = osb_state[rel]
                nc.scalar.activation(
                    osb[:, oj, :], po[:], act_fn,
                    bias=0.0, scale=rs_t[:, tglob:tglob + 1], alpha=0.01)
                if oj == OUT_GRP - 1 or t == ntiles - 1:
                    cnt = t - osb_t0 + 1
                    g = osb_t0 // OUT_GRP
                    dst = out_d[rel].ap()[g * 128:(g + 1) * 128, :cnt * HID]
                    nc.sync.dma_start(
                        dst.rearrange("p (j k) -> p j k", k=HID),
                        osb[:, :cnt, :])

    nc.compile()
    return nc


def _run(nc, in_maps, trace=False, **kw):
    from concourse import bass_utils
    res = bass_utils.run_bass_kernel_spmd(
        nc, in_maps, core_ids=list(range(NCORES)), trace=trace, **kw)
    return res


def _assemble(results, meta):
    out = np.empty((NODE_N + INST_N + SVC_N, HID), np.float32)
    offs = [0, NODE_N, NODE_N + INST_N]
    names = ["out_node", "out_inst", "out_svc"]
    for rel in range(3):
        nt = meta["ntiles"][rel]
        ngrp = _cdiv(nt, OUT_GRP)
        for c in range(NCORES):
            ids = meta["dst_ids"][rel][c]
            m = ids >= 0
            arr = np.asarray(results[c][names[rel]], np.float32)
            rows = arr.reshape(ngrp, 128, OUT_GRP, HID).transpose(
                0, 2, 1, 3).reshape(ngrp * OUT_GRP * 128, HID)[:nt * 128]
            out[offs[rel] + ids[m]] = rows[m]
    return out


def kernel(**inputs):
    import hashlib
    key = "prog"
    h = hashlib.sha1()
    for k in ("sc_src", "sc_dst", "in_src", "in_dst", "ni_src", "ni_dst"):
        h.update(np.ascontiguousarray(np.asarray(inputs[k], np.int32)).tobytes())
    sig = h.hexdigest()
    meta, in_maps = _build_host_data(inputs)
    if key in _cache and _cache[key][0] == sig:
        _, nc, _ = _cache[key]
    else:
        nc = _build_program(meta)
        _cache[key] = (sig, nc, meta)
    res = _run(nc, in_maps)
    return _assemble(res.results, meta)


# revision 42
# speedup vs baseline: 1.0239x; 1.0239x over previous
"""Trainium2 Bass kernel for a heterogeneous GraphConv layer (3 relations).

out = concat([leaky(GC(inst_feat, W_inst, in_*)),     # -> node   (10000)
              leaky(GC(node_feat, W_node, ni_*)),     # -> inst   (100000)
              leaky(GC(svc_feat,  W_svc,  sc_*))])    # -> svc    (20000)

GC(f, W, src, dst) = rsqrt(deg_d) * segsum_dst((rsqrt(deg_s)*f)[src]) @ W + b
(aggregation commutes with the dense @W, so we gather *raw scaled features*
and apply W once per destination tile).

Strategy: destination-sharded across 8 NeuronCores, with host-balanced
dst->(core,tile,slot) assignment (tile edge sums packed to multiples of 128
and rank-matched across cores so the shared program's per-tile block counts
stay near the mean).

DMA-gather cost on TRN2 is per-descriptor: a 512B descriptor costs the same
as 256B (sub-512B transfers pay a 2x bus penalty), so the gather tables are
laid out as PAIR rows [2*128] fp16 = 512B: one descriptor fetches two
feature rows. The per-core table is ordered by first use so the two edges
that introduce a pair of new sources in the same dst tile share one
descriptor ("paired slots"); repeat edges use one half and the other half
rides free (and occasionally serves a second edge whose source lands on the
pair sibling). Per slot there are two one-hot channels dlA/dlB (-1 = unused)
selecting the dst column for the low/high half row.

Device work per 128-slot block: dma_gather 128 pair rows -> [128e, 256f],
DVE tensor_scalar builds one-hot stA[e,d]=(iota==dlA) (and stB for the
pair-slot prefix blocks only), PE accumulates aggT[f,d] += A.T@stA (+B.T@stB)
in PSUM. Per dst tile: PSUM out = u (x) b (rank-1 K=1 fp32 matmul preloading
bias) + aggT.T @ W (fp16), then one ScalarE Lrelu(out * rsqrt_deg_d) and a
grouped fp16 DMA to the output rows.
"""

import os as _os

import numpy as np

SVC_N, INST_N, NODE_N, HID = 20000, 100000, 10000, 128
NCORES = 8
BLK = 128           # slots per one-hot matmul block
# blocks per dma_gather instruction. NOTE: needs single_packet=False — with
# single_packet=True the whole stream coalesces into one DMA packet, which
# caps at 64 descriptors/engine (num_idxs <= 1024); beyond that the exec
# unit faults (NRT_EXEC_UNIT_UNRECOVERABLE).
CHUNK = int(_os.environ.get("GNN_CHUNK", "16"))
GDT = "fp16"
ACT_MODE = "lrelu"  # "lrelu" (HW leaky relu) | "relu" (sim debug)
PAIR = 2 * HID      # table row = pair of feature rows (512B fp16)
# dst tiles batched per epilogue staging buffer / out DMA
OUT_GRP = int(_os.environ.get("GNN_OUT_GRP", "8"))

_cache = {}


def _cdiv(a, b):
    return (a + b - 1) // b


def _rup(a, b):
    return _cdiv(a, b) * b


def _balanced_assign(deg, n_dst, ntiles, rho):
    """Assign dst nodes to (core, tile, slot) packing per-tile SLOT counts
    (estimated as edges*(1-rho), rho = relation merge rate) just under
    multiples of 128 so per-tile block counts carry minimal ceil padding.

    Returns dst_ids[NCORES, ntiles*128] int64 (-1 = pad slot): the global dst
    node stored at each (core, tile, slot).
    """
    order = np.argsort(-deg, kind="stable")
    # snake over cores -> per-core totals equal to within one max-degree
    core_of = np.empty(n_dst, np.int64)
    snake = np.concatenate([np.arange(NCORES), np.arange(NCORES)[::-1]])
    core_of[order] = snake[np.arange(n_dst) % (2 * NCORES)]

    dst_ids = np.full((NCORES, ntiles * 128), -1, np.int64)
    for c in range(NCORES):
        ids = order[core_of[order] == c]  # degree-descending
        w = deg[ids]
        total = int(w.sum())
        # uniform edge-weight target per bin (multiple of 128; rho reserved
        # for a future slot-aware quota scheme — measured merge-rate noise
        # and the max-over-core coupling made per-bin quotas regress)
        target = np.full(ntiles, _cdiv(total, ntiles * 128) * 128, np.int64)
        binw = np.zeros(ntiles, np.int64)
        binn = np.zeros(ntiles, np.int64)
        bins = [[] for _ in range(ntiles)]
        # greedy: place each dst (deg desc) in the fullest bin it still fits
        # (by weight target and 128-slot cap); else least-filled open bin
        open_bins = list(range(ntiles))
        for i, d in zip(ids, w):
            best, bestw = -1, -1
            for t in open_bins:
                if binw[t] + d <= target[t] and binw[t] > bestw:
                    best, bestw = t, binw[t]
            if best < 0:
                best = min(open_bins, key=lambda x: binw[x])
            t = best
            bins[t].append(i)
            binw[t] += d
            binn[t] += 1
            if binn[t] >= 128:
                open_bins.remove(t)
        for t in range(ntiles):
            ids_t = bins[t]
            dst_ids[c, t * 128: t * 128 + len(ids_t)] = ids_t
    return dst_ids


def _pack_core(es, ed, n_src, ntiles):
    """Pair-slot packing for one (core, relation).

    es: edge source node ids; ed: edge dst slot (tile*128 + dst_local).

    The gather table holds one 512B row per USED source: [feat(r), feat(r+1)]
    (staggered duplicate), so descriptor idx r serves edge(s) on table row r
    via channel A and optionally a second edge on row r+1 via channel B.
    Rows are ordered by their tile-usage lists (lexsort) so edges of the same
    tile sit on adjacent rows and merge into shared slots.

    Returns dict with: table_rows (src id per table row), per-tile slot
    arrays (row idx, dlA, dlB), nslot[t], nB[t].
    """
    KEYLEN = 6
    tile_of = (ed >> 7).astype(np.int64)
    dl = (ed & 127).astype(np.int64)

    rows_used = np.unique(es)
    nrows = len(rows_used)
    rid_of = np.full(n_src, -1, np.int64)
    rid_of[rows_used] = np.arange(nrows)

    # tile-usage key per row: first KEYLEN tiles (sorted), padded
    pt = np.unique(np.stack([rid_of[es], tile_of], axis=1), axis=0)
    grp_new = np.r_[True, pt[1:, 0] != pt[:-1, 0]]
    idx = np.arange(len(pt))
    j = idx - np.maximum.accumulate(np.where(grp_new, idx, 0))
    keymat = np.full((nrows, KEYLEN), 32767, np.int64)
    m = j < KEYLEN
    keymat[pt[m, 0], j[m]] = pt[m, 1]
    order = np.lexsort(keymat.T[::-1])
    table_rows = rows_used[order]          # src id at each table position
    pos_of = np.full(n_src, -1, np.int64)
    pos_of[table_rows] = np.arange(nrows)

    r = pos_of[es]
    # per tile: sort edges by table position; pair edges on consecutive
    # positions (runs split on gaps/duplicates, paired (0,1)(2,3)... in-run)
    o = np.lexsort((r, tile_of))
    kt, kr, kdl = tile_of[o], r[o], dl[o]
    brk = np.r_[True, (kt[1:] != kt[:-1]) | (kr[1:] != kr[:-1] + 1)]
    idx = np.arange(len(kt))
    k_in_run = idx - np.maximum.accumulate(np.where(brk, idx, 0))
    run_id = np.cumsum(brk) - 1
    half = k_in_run & 1
    skey = np.stack([run_id, k_in_run >> 1], axis=1)
    uslot, inv = np.unique(skey, axis=0, return_inverse=True)
    ns = len(uslot)
    s_tile = np.zeros(ns, np.int64)
    s_row = np.zeros(ns, np.int64)
    dlA = np.full(ns, -1, np.int64)
    dlB = np.full(ns, -1, np.int64)
    m0 = half == 0
    s_tile[inv[m0]] = kt[m0]
    s_row[inv[m0]] = kr[m0]
    dlA[inv[m0]] = kdl[m0]
    dlB[inv[~m0]] = kdl[~m0]
    has_b = dlB >= 0

    # order slots per tile: B-present first (so stB/matmul-B run only on a
    # prefix of blocks), then by row for gather locality
    so = np.lexsort((s_row, ~has_b, s_tile))
    s_tile, s_row, dlA, dlB, has_b = (
        s_tile[so], s_row[so], dlA[so], dlB[so], has_b[so])

    nslot = np.bincount(s_tile, minlength=ntiles)
    nB = np.bincount(s_tile[has_b], minlength=ntiles)
    tstart = np.r_[0, np.cumsum(nslot)]
    tiles = []
    for t in range(ntiles):
        sl = slice(tstart[t], tstart[t + 1])
        tiles.append((s_row[sl], dlA[sl], dlB[sl]))
    return dict(table_rows=table_rows, tiles=tiles, nslot=nslot, nB=nB)


def _prep_relation(src, dst, n_src, n_dst, feat, rho):
    """Host-side sharding/packing for one relation.

    rho: estimated slot merge rate (pair-served edge fraction) used to pack
    tiles to near-multiple-of-128 slot counts.
    """
    src = np.asarray(src, np.int64)
    dst = np.asarray(dst, np.int64)
    deg_s = np.maximum(np.bincount(src, minlength=n_src), 1).astype(np.float64)
    deg_d_raw = np.bincount(dst, minlength=n_dst)
    deg_d = np.maximum(deg_d_raw, 1).astype(np.float64)
    rs_s = (1.0 / np.sqrt(deg_s)).astype(np.float32)
    rs_d = (1.0 / np.sqrt(deg_d)).astype(np.float32)
    u_d = np.sqrt(deg_d).astype(np.float32)  # ~= 1/rs_d

    feat_s = (np.asarray(feat, np.float32) * rs_s[:, None]).astype(np.float32)

    D = _rup(_cdiv(n_dst, NCORES), 128)  # dst rows per core (padded)
    ntiles = D // 128

    dst_ids = _balanced_assign(deg_d_raw.astype(np.int64), n_dst, ntiles, rho)
    slot_core = np.empty(n_dst, np.int64)
    slot_loc = np.empty(n_dst, np.int64)
    for c in range(NCORES):
        m = dst_ids[c] >= 0
        slot_core[dst_ids[c, m]] = c
        slot_loc[dst_ids[c, m]] = np.nonzero(m)[0]

    e_core = slot_core[dst]
    e_loc = slot_loc[dst]
    cores = []
    for c in range(NCORES):
        m = e_core == c
        pk = _pack_core(src[m], e_loc[m], n_src, ntiles)
        pk["dst_ids"] = dst_ids[c].copy()
        cores.append(pk)

    # rank-match: per core sort its tiles by slot count desc so tile index t
    # has similar (max-over-core) block counts
    for c in range(NCORES):
        pk = cores[c]
        perm = np.argsort(-pk["nslot"], kind="stable")
        pk["tiles"] = [pk["tiles"][t] for t in perm]
        pk["nslot"] = pk["nslot"][perm]
        pk["nB"] = pk["nB"][perm]
        pk["dst_ids"] = pk["dst_ids"].reshape(ntiles, 128)[perm].reshape(-1)

    nslot_all = np.stack([cores[c]["nslot"] for c in range(NCORES)])
    nB_all = np.stack([cores[c]["nB"] for c in range(NCORES)])
    btile = np.maximum(_cdiv(np.max(nslot_all, axis=0), BLK), 1)
    nbtile = np.minimum(_cdiv(np.max(nB_all, axis=0), BLK), btile)
    nrows = max(len(cores[c]["table_rows"]) for c in range(NCORES))

    return dict(cores=cores, btile=btile, nbtile=nbtile, nrows=nrows,
                nblk=int(btile.sum()), ntiles=ntiles, D=D,
                feat_s=feat_s, rs_d=rs_d, u_d=u_d, n_dst=n_dst)


def _build_host_data(inputs):
    rels = [
        # order matters: output rows are [node_out, inst_out, svc_out].
        # rho = measured pair-merge rate per relation on this graph
        _prep_relation(inputs["in_src"], inputs["in_dst"], INST_N, NODE_N,
                       inputs["instance_feat"], rho=0.49),
        _prep_relation(inputs["ni_src"], inputs["ni_dst"], NODE_N, INST_N,
                       inputs["node_feat"], rho=0.30),
        _prep_relation(inputs["sc_src"], inputs["sc_dst"], SVC_N, SVC_N,
                       inputs["svc_feat"], rho=0.41),
    ]
    Ws = [inputs["W_inst"], inputs["W_node"], inputs["W_svc"]]
    bs = [inputs["b_inst"], inputs["b_node"], inputs["b_svc"]]

    nblk_pads = [_rup(r["nblk"], CHUNK) for r in rels]
    nblk_tot = sum(nblk_pads)
    nidx_tot = nblk_tot * BLK
    ntile_tot = sum(r["ntiles"] for r in rels)

    W_cat = np.concatenate([np.asarray(w, np.float32) for w in Ws],
                           axis=1).astype(np.float16)
    b_row = np.concatenate([np.asarray(b, np.float32) for b in bs]
                           )[None, :].astype(np.float16)
    iota = np.tile(np.arange(128, dtype=np.float32), (128, 1)).astype(np.float16)

    in_maps = []
    for c in range(NCORES):
        gidx = np.full(nidx_tot, -1, np.int64)
        dlA = np.full(nidx_tot, -1.0, np.float32)
        dlB = np.full(nidx_tot, -1.0, np.float32)
        tbls = []
        off = 0  # block offset in global stream
        for r in rels:
            pk = r["cores"][c]
            assert r["nrows"] < 32768, "row idx must fit int16"
            # staggered-duplicate pair rows: tbl[i] = [feat(i), feat(i+1)]
            fr = np.zeros((r["nrows"] + 1, HID), np.float16)
            tr = pk["table_rows"]
            fr[:len(tr)] = r["feat_s"][tr].astype(np.float16)
            tbl = np.concatenate([fr[:-1], fr[1:]], axis=1)
            tbls.append(np.ascontiguousarray(tbl))
            for t in range(r["ntiles"]):
                sp, da, db = pk["tiles"][t]
                n = len(sp)
                base = off * BLK
                gidx[base:base + n] = sp
                gidx[base + n: base + int(r["btile"][t]) * BLK] = 0
                dlA[base:base + n] = da
                dlB[base:base + n] = db
                off += int(r["btile"][t])
            # relation-tail pad blocks keep idx -1 (trimmed device-side)
            off = _rup(off, CHUNK)

        idx16 = np.ascontiguousarray(
            gidx.astype(np.int16).reshape(-1, 16).T)
        idx_sb = np.tile(idx16, (8, 1))                          # [128, nidx/16]
        dlA_sb = np.ascontiguousarray(dlA.reshape(nblk_tot, BLK).T)
        dlB_sb = np.ascontiguousarray(dlB.reshape(nblk_tot, BLK).T)

        rs_sb = np.zeros((128, ntile_tot), np.float32)
        u_sb = np.zeros((1, ntile_tot * 128), np.float32)
        t0 = 0
        for r in rels:
            ids = r["cores"][c]["dst_ids"]
            val_rs = np.zeros(r["D"], np.float32)
            val_u = np.zeros(r["D"], np.float32)
            m = ids >= 0
            val_rs[m] = r["rs_d"][ids[m]]
            val_u[m] = r["u_d"][ids[m]]
            rs_sb[:, t0:t0 + r["ntiles"]] = val_rs.reshape(r["ntiles"], 128).T
            u_sb[0, t0 * 128:(t0 + r["ntiles"]) * 128] = val_u
            t0 += r["ntiles"]
        u_sb = u_sb.astype(np.float16)

        in_maps.append({
            "tbl_in": tbls[0],
            "tbl_ni": tbls[1],
            "tbl_sc": tbls[2],
            "idx_sb": np.ascontiguousarray(idx_sb),
            "dlA_sb": dlA_sb,
            "dlB_sb": dlB_sb,
            "rs_sb": rs_sb,
            "u_sb": u_sb,
            "W_cat": np.ascontiguousarray(W_cat),
            "b_row": np.ascontiguousarray(b_row),
            "iota": np.ascontiguousarray(iota),
        })

    meta = dict(
        nblk_tot=nblk_tot, nidx_tot=nidx_tot, ntile_tot=ntile_tot,
        nrowss=[r["nrows"] for r in rels],
        btiles=[r["btile"].tolist() for r in rels],
        nbtiles=[r["nbtile"].tolist() for r in rels],
        ntiles=[r["ntiles"] for r in rels],
        Ds=[r["D"] for r in rels],
        n_dsts=[r["n_dst"] for r in rels],
        dst_ids=[[r["cores"][c]["dst_ids"] for c in range(NCORES)]
                 for r in rels],
    )
    return meta, in_maps


def _build_program(meta):
    import os

    import concourse.bacc as bacc
    import concourse.mybir as mybir
    import concourse.tile as tile

    dbg_max_tiles = int(os.environ.get("GNN_MAX_TILES", "0"))  # 0 = all

    gdt = mybir.dt.float16
    f16 = mybir.dt.float16
    f32 = mybir.dt.float32
    AF = mybir.ActivationFunctionType
    act_fn = AF.Lrelu if ACT_MODE == "lrelu" else AF.Relu

    nblk_tot, nidx_tot, ntile_tot = (meta["nblk_tot"], meta["nidx_tot"],
                                     meta["ntile_tot"])

    nc = bacc.Bacc("TRN2", target_bir_lowering=False, debug=False,
                   enable_asserts=False, num_devices=NCORES)

    tbl_d = [
        nc.dram_tensor("tbl_in", [meta["nrowss"][0], PAIR], gdt,
                       kind="ExternalInput"),
        nc.dram_tensor("tbl_ni", [meta["nrowss"][1], PAIR], gdt,
                       kind="ExternalInput"),
        nc.dram_tensor("tbl_sc", [meta["nrowss"][2], PAIR], gdt,
                       kind="ExternalInput"),
    ]
    idx_d = nc.dram_tensor("idx_sb", [128, nidx_tot // 16], mybir.dt.int16,
                           kind="ExternalInput")
    dlA_d = nc.dram_tensor("dlA_sb", [128, nblk_tot], f32, kind="ExternalInput")
    dlB_d = nc.dram_tensor("dlB_sb", [128, nblk_tot], f32, kind="ExternalInput")
    rs_d = nc.dram_tensor("rs_sb", [128, ntile_tot], f32, kind="ExternalInput")
    u_d = nc.dram_tensor("u_sb", [1, ntile_tot * 128], f16, kind="ExternalInput")
    W_d = nc.dram_tensor("W_cat", [128, 3 * HID], f16, kind="ExternalInput")
    b_d = nc.dram_tensor("b_row", [1, 3 * HID], f16, kind="ExternalInput")
    iota_d = nc.dram_tensor("iota", [128, 128], gdt, kind="ExternalInput")

    # p-major grouped layout: row g*128+p holds OUT_GRP tiles' rows for dst
    # slot p — out DMA descriptors are OUT_GRP*256B contiguous (no sub-512B
    # DMA bus penalty); host assemble untangles
    out_d = [
        nc.dram_tensor(n, [_cdiv(meta["ntiles"][i], OUT_GRP) * 128,
                           OUT_GRP * HID], f16, kind="ExternalOutput")
        for i, n in enumerate(["out_node", "out_inst", "out_svc"])
    ]

    with tile.TileContext(nc) as tc:
        with (
            tc.tile_pool(name="const", bufs=1) as const,
            tc.tile_pool(name="g", bufs=int(os.environ.get("GNN_GBUFS", "6"))) as gpool,
            tc.tile_pool(name="st", bufs=int(os.environ.get("GNN_STBUFS", "48"))) as stpool,
            tc.tile_pool(name="evac", bufs=int(os.environ.get("GNN_EVBUFS", "6"))) as evac,
            tc.tile_pool(name="osb", bufs=int(os.environ.get("GNN_OBUFS", "6"))) as opool,
            tc.tile_pool(name="psA", bufs=5, space="PSUM") as psA,
            tc.tile_pool(name="psO", bufs=3, space="PSUM") as psO,
        ):
            rel_nblks = [sum(meta["btiles"][r]) for r in range(3)]
            rel_base = [0, 0, 0]  # global block base per relation
            for r in range(1, 3):
                rel_base[r] = rel_base[r - 1] + _rup(rel_nblks[r - 1], CHUNK)
            tg_base = [0, meta["ntiles"][0],
                       meta["ntiles"][0] + meta["ntiles"][1]]

            # first gathers depend only on the leading idx slices + dl/iota:
            # load those first so the gather stream starts ASAP (HWDGE is
            # FIFO per issuing engine)
            idx_t = const.tile([128, nidx_tot // 16], mybir.dt.int16)
            heads = []
            for r in range(3):
                h0 = rel_base[r] * BLK // 16
                h1 = min(h0 + 2 * CHUNK * BLK // 16, nidx_tot // 16)
                heads.append((h0, h1))
                nc.sync.dma_start(idx_t[:, h0:h1], idx_d.ap()[:, h0:h1])
            dlA_t = const.tile([128, nblk_tot], f32)
            nc.sync.dma_start(dlA_t[:], dlA_d.ap())
            dlB_t = const.tile([128, nblk_tot], f32)
            nc.sync.dma_start(dlB_t[:], dlB_d.ap())
            iota_t = const.tile([128, 128], gdt)
            nc.sync.dma_start(iota_t[:], iota_d.ap())
            W_t = const.tile([128, 3 * HID], f16)
            nc.sync.dma_start(W_t[:], W_d.ap())
            b_t = const.tile([1, 3 * HID], f16)
            nc.sync.dma_start(b_t[:], b_d.ap())
            u_t = const.tile([1, ntile_tot * 128], f16)
            nc.sync.dma_start(u_t[:], u_d.ap())
            rs_t = const.tile([128, ntile_tot], f32)
            nc.sync.dma_start(rs_t[:], rs_d.ap())
            for r in range(3):
                h1 = heads[r][1]
                end = rel_base[r] * BLK // 16 + \
                    _rup(rel_nblks[r], CHUNK) * BLK // 16
                if h1 < end:
                    nc.sync.dma_start(idx_t[:, h1:end], idx_d.ap()[:, h1:end])

            g_tiles = {}   # global chunk id -> gather tile

            def issue_gather(rel, lci):
                gci = rel_base[rel] // CHUNK + lci
                gt = gpool.tile([128, CHUNK, PAIR], gdt, tag="g")
                nidx = CHUNK * BLK
                # trailing -1 idxs (relation-tail pads) are skipped; trim reg
                real_blocks = max(0, min(CHUNK, rel_nblks[rel] - lci * CHUNK))
                nc.gpsimd.dma_gather(
                    out_ap=gt[:],
                    in_ap=tbl_d[rel].ap(),
                    idxs_ap=idx_t[:, gci * (nidx // 16):(gci + 1) * (nidx // 16)],
                    num_idxs=nidx,
                    num_idxs_reg=max(BLK, real_blocks * BLK),
                    elem_size=PAIR,
                    single_packet=False,
                )
                g_tiles[gci] = gt

            # relation-interleaved tile schedule: spread the epilogue-heavy
            # relation (ni: many tiles, few blocks) evenly across the
            # gather-heavy one (sc) so no engine's work bunches up
            sched = []
            scale = [0.93, 0.95, 1.0]  # in/ni finish early; sc's last blocks
            for r in range(3):         # keep gathers flowing during drain
                btile = meta["btiles"][r]
                o = 0
                for t in range(meta["ntiles"][r]):
                    if dbg_max_tiles and t >= dbg_max_tiles:
                        break
                    # key on end-fraction: the block-heavy relation's last
                    # tile keeps gathers flowing while light tiles' epilogues
                    # drain, shrinking the no-DMA tail
                    sched.append((scale[r] * (o + btile[t])
                                  / max(1, rel_nblks[r]), r, t))
                    o += btile[t]
            sched.sort()

            rel_blk = [0, 0, 0]       # relation-local block cursor
            osb_state = [None, None, None]

            for _, rel, t in sched:
                ntiles = meta["ntiles"][rel]
                nb = meta["btiles"][rel][t]
                nbB = meta["nbtiles"][rel][t]
                agg = psA.tile([128, 128], f32, tag="agg")
                for b in range(nb):
                    lb = rel_blk[rel]
                    lci, cj = divmod(lb, CHUNK)
                    if cj == 0:
                        issue_gather(rel, lci)
                    gci = rel_base[rel] // CHUNK + lci
                    blk = rel_base[rel] + lb  # global dl column
                    has_b = b < nbB
                    stA = stpool.tile([128, 128], gdt, tag="stA")
                    nc.vector.tensor_scalar(
                        stA[:], iota_t[:], dlA_t[:, blk:blk + 1], None,
                        mybir.AluOpType.is_equal)
                    last = b == nb - 1
                    nc.tensor.matmul(
                        agg[:], g_tiles[gci][:, cj, 0:HID], stA[:],
                        start=(b == 0), stop=(last and not has_b))
                    if has_b:
                        stB = stpool.tile([128, 128], gdt, tag="stB")
                        nc.vector.tensor_scalar(
                            stB[:], iota_t[:], dlB_t[:, blk:blk + 1], None,
                            mybir.AluOpType.is_equal)
                        nc.tensor.matmul(
                            agg[:], g_tiles[gci][:, cj, HID:PAIR], stB[:],
                            start=False, stop=last)
                    rel_blk[rel] += 1
                tglob = tg_base[rel] + t
                aggsb = evac.tile([128, 128], f16, tag="evac")
                nc.scalar.copy(aggsb[:], agg[:])
                po = psO.tile([128, 128], f32, tag="po")
                nc.tensor.matmul(
                    po[:], u_t[:, tglob * 128:(tglob + 1) * 128],
                    b_t[:, rel * HID:(rel + 1) * HID],
                    start=True, stop=False, skip_group_check=True)
                nc.tensor.matmul(
                    po[:], aggsb[:], W_t[:, rel * HID:(rel + 1) * HID],
                    start=False, stop=True, skip_group_check=True)
                oj = t % OUT_GRP
                if oj == 0:
                    osb_state[rel] = (
                        opool.tile([128, OUT_GRP, 128], f16, tag="osb",
                                   name="osb"), t)
                osb, osb_t0 = osb_state[rel]
                nc.scalar.activation(
                    osb[:, oj, :], po[:], act_fn,
                    bias=0.0, scale=rs_t[:, tglob:tglob + 1], alpha=0.01)
                if oj == OUT_GRP - 1 or t == ntiles - 1:
                    cnt = t - osb_t0 + 1
                    g = osb_t0 // OUT_GRP
                    dst = out_d[rel].ap()[g * 128:(g + 1) * 128, :cnt * HID]
                    nc.sync.dma_start(
                        dst.rearrange("p (j k) -> p j k", k=HID),
                        osb[:, :cnt, :])

    nc.compile()
    return nc


def _run(nc, in_maps, trace=False, **kw):
    from concourse import bass_utils
    res = bass_utils.run_bass_kernel_spmd(
        nc, in_maps, core_ids=list(range(NCORES)), trace=trace, **kw)
    return res


def _assemble(results, meta):
    out = np.empty((NODE_N + INST_N + SVC_N, HID), np.float32)
    offs = [0, NODE_N, NODE_N + INST_N]
    names = ["out_node", "out_inst", "out_svc"]
    for rel in range(3):
        nt = meta["ntiles"][rel]
        ngrp = _cdiv(nt, OUT_GRP)
        for c in range(NCORES):
            ids = meta["dst_ids"][rel][c]
            m = ids >= 0
            arr = np.asarray(results[c][names[rel]], np.float32)
            rows = arr.reshape(ngrp, 128, OUT_GRP, HID).transpose(
                0, 2, 1, 3).reshape(ngrp * OUT_GRP * 128, HID)[:nt * 128]
            out[offs[rel] + ids[m]] = rows[m]
    return out


def kernel(**inputs):
    import hashlib
    key = "prog"
    h = hashlib.sha1()
    for k in ("sc_src", "sc_dst", "in_src", "in_dst", "ni_src", "ni_dst"):
        h.update(np.ascontiguousarray(np.asarray(inputs[k], np.int32)).tobytes())
    sig = h.hexdigest()
    meta, in_maps = _build_host_data(inputs)
    if key in _cache and _cache[key][0] == sig:
        _, nc, _ = _cache[key]
    else:
        nc = _build_program(meta)
        _cache[key] = (sig, nc, meta)
    res = _run(nc, in_maps)
    return _assemble(res.results, meta)


# revision 52
# speedup vs baseline: 1.0450x; 1.0206x over previous
"""Trainium2 Bass kernel for a heterogeneous GraphConv layer (3 relations).

out = concat([leaky(GC(inst_feat, W_inst, in_*)),     # -> node   (10000)
              leaky(GC(node_feat, W_node, ni_*)),     # -> inst   (100000)
              leaky(GC(svc_feat,  W_svc,  sc_*))])    # -> svc    (20000)

GC(f, W, src, dst) = rsqrt(deg_d) * segsum_dst((rsqrt(deg_s)*f)[src]) @ W + b
(aggregation commutes with the dense @W, so we gather *raw scaled features*
and apply W once per destination tile).

Strategy: destination-sharded across 8 NeuronCores, with host-balanced
dst->(core,tile,slot) assignment (tile edge sums packed to multiples of 128
and rank-matched across cores so the shared program's per-tile block counts
stay near the mean).

DMA-gather cost on TRN2 is per-descriptor: a 512B descriptor costs the same
as 256B (sub-512B transfers pay a 2x bus penalty), so the gather tables are
laid out as PAIR rows [2*128] fp16 = 512B: one descriptor fetches two
feature rows. The per-core table is ordered by first use so the two edges
that introduce a pair of new sources in the same dst tile share one
descriptor ("paired slots"); repeat edges use one half and the other half
rides free (and occasionally serves a second edge whose source lands on the
pair sibling). Per slot there are two one-hot channels dlA/dlB (-1 = unused)
selecting the dst column for the low/high half row.

Device work per 128-slot block: dma_gather 128 pair rows -> [128e, 256f],
DVE tensor_scalar builds one-hot stA[e,d]=(iota==dlA) (and stB for the
pair-slot prefix blocks only), PE accumulates aggT[f,d] += A.T@stA (+B.T@stB)
in PSUM. Per dst tile: PSUM out = u (x) b (rank-1 K=1 fp32 matmul preloading
bias) + aggT.T @ W (fp16), then one ScalarE Lrelu(out * rsqrt_deg_d) and a
grouped fp16 DMA to the output rows.
"""

import os as _os

import numpy as np

SVC_N, INST_N, NODE_N, HID = 20000, 100000, 10000, 128
NCORES = 8
BLK = 128           # slots per one-hot matmul block
# blocks per dma_gather instruction. NOTE: needs single_packet=False — with
# single_packet=True the whole stream coalesces into one DMA packet, which
# caps at 64 descriptors/engine (num_idxs <= 1024); beyond that the exec
# unit faults (NRT_EXEC_UNIT_UNRECOVERABLE).
CHUNK = int(_os.environ.get("GNN_CHUNK", "16"))
GDT = "fp16"
ACT_MODE = "lrelu"  # "lrelu" (HW leaky relu) | "relu" (sim debug)
PAIR = 2 * HID      # table row = pair of feature rows (512B fp16)
# dst tiles batched per epilogue staging buffer / out DMA
OUT_GRP = int(_os.environ.get("GNN_OUT_GRP", "8"))

_cache = {}


def _cdiv(a, b):
    return (a + b - 1) // b


def _rup(a, b):
    return _cdiv(a, b) * b


def _balanced_assign(deg, n_dst, ntiles, rho):
    """Assign dst nodes to (core, tile, slot) packing per-tile SLOT counts
    (estimated as edges*(1-rho), rho = relation merge rate) just under
    multiples of 128 so per-tile block counts carry minimal ceil padding.

    Returns dst_ids[NCORES, ntiles*128] int64 (-1 = pad slot): the global dst
    node stored at each (core, tile, slot).
    """
    order = np.argsort(-deg, kind="stable")
    # snake over cores -> per-core totals equal to within one max-degree
    core_of = np.empty(n_dst, np.int64)
    snake = np.concatenate([np.arange(NCORES), np.arange(NCORES)[::-1]])
    core_of[order] = snake[np.arange(n_dst) % (2 * NCORES)]

    dst_ids = np.full((NCORES, ntiles * 128), -1, np.int64)
    for c in range(NCORES):
        ids = order[core_of[order] == c]  # degree-descending
        w = deg[ids]
        total = int(w.sum())
        # uniform edge-weight target per bin (multiple of 128; rho reserved
        # for a future slot-aware quota scheme — measured merge-rate noise
        # and the max-over-core coupling made per-bin quotas regress)
        target = np.full(ntiles, _cdiv(total, ntiles * 128) * 128, np.int64)
        binw = np.zeros(ntiles, np.int64)
        binn = np.zeros(ntiles, np.int64)
        bins = [[] for _ in range(ntiles)]
        # greedy: place each dst (deg desc) in the fullest bin it still fits
        # (by weight target and 128-slot cap); else least-filled open bin
        open_bins = list(range(ntiles))
        for i, d in zip(ids, w):
            best, bestw = -1, -1
            for t in open_bins:
                if binw[t] + d <= target[t] and binw[t] > bestw:
                    best, bestw = t, binw[t]
            if best < 0:
                best = min(open_bins, key=lambda x: binw[x])
            t = best
            bins[t].append(i)
            binw[t] += d
            binn[t] += 1
            if binn[t] >= 128:
                open_bins.remove(t)
        for t in range(ntiles):
            ids_t = bins[t]
            dst_ids[c, t * 128: t * 128 + len(ids_t)] = ids_t
    return dst_ids


def _pack_core(es, ed, n_src, ntiles):
    """Pair-slot packing for one (core, relation).

    es: edge source node ids; ed: edge dst slot (tile*128 + dst_local).

    The gather table holds one 512B row per USED source: [feat(r), feat(r+1)]
    (staggered duplicate), so descriptor idx r serves edge(s) on table row r
    via channel A and optionally a second edge on row r+1 via channel B.
    Rows are ordered by their tile-usage lists (lexsort) so edges of the same
    tile sit on adjacent rows and merge into shared slots.

    Returns dict with: table_rows (src id per table row), per-tile slot
    arrays (row idx, dlA, dlB), nslot[t], nB[t].
    """
    KEYLEN = 6
    tile_of = (ed >> 7).astype(np.int64)
    dl = (ed & 127).astype(np.int64)

    rows_used = np.unique(es)
    nrows = len(rows_used)
    rid_of = np.full(n_src, -1, np.int64)
    rid_of[rows_used] = np.arange(nrows)

    # tile-usage key per row: first KEYLEN tiles (sorted), padded
    pt = np.unique(np.stack([rid_of[es], tile_of], axis=1), axis=0)
    grp_new = np.r_[True, pt[1:, 0] != pt[:-1, 0]]
    idx = np.arange(len(pt))
    j = idx - np.maximum.accumulate(np.where(grp_new, idx, 0))
    keymat = np.full((nrows, KEYLEN), 32767, np.int64)
    m = j < KEYLEN
    keymat[pt[m, 0], j[m]] = pt[m, 1]
    order = np.lexsort(keymat.T[::-1])
    table_rows = rows_used[order]          # src id at each table position
    pos_of = np.full(n_src, -1, np.int64)
    pos_of[table_rows] = np.arange(nrows)

    r = pos_of[es]
    # per tile: sort edges by table position; pair edges on consecutive
    # positions (runs split on gaps/duplicates, paired (0,1)(2,3)... in-run)
    o = np.lexsort((r, tile_of))
    kt, kr, kdl = tile_of[o], r[o], dl[o]
    brk = np.r_[True, (kt[1:] != kt[:-1]) | (kr[1:] != kr[:-1] + 1)]
    idx = np.arange(len(kt))
    k_in_run = idx - np.maximum.accumulate(np.where(brk, idx, 0))
    run_id = np.cumsum(brk) - 1
    half = k_in_run & 1
    skey = np.stack([run_id, k_in_run >> 1], axis=1)
    uslot, inv = np.unique(skey, axis=0, return_inverse=True)
    ns = len(uslot)
    s_tile = np.zeros(ns, np.int64)
    s_row = np.zeros(ns, np.int64)
    dlA = np.full(ns, -1, np.int64)
    dlB = np.full(ns, -1, np.int64)
    m0 = half == 0
    s_tile[inv[m0]] = kt[m0]
    s_row[inv[m0]] = kr[m0]
    dlA[inv[m0]] = kdl[m0]
    dlB[inv[~m0]] = kdl[~m0]
    has_b = dlB >= 0

    # order slots per tile: B-present first (so stB/matmul-B run only on a
    # prefix of blocks), then by row for gather locality
    so = np.lexsort((s_row, ~has_b, s_tile))
    s_tile, s_row, dlA, dlB, has_b = (
        s_tile[so], s_row[so], dlA[so], dlB[so], has_b[so])

    nslot = np.bincount(s_tile, minlength=ntiles)
    nB = np.bincount(s_tile[has_b], minlength=ntiles)
    tstart = np.r_[0, np.cumsum(nslot)]
    tiles = []
    for t in range(ntiles):
        sl = slice(tstart[t], tstart[t + 1])
        tiles.append((s_row[sl], dlA[sl], dlB[sl]))
    return dict(table_rows=table_rows, tiles=tiles, nslot=nslot, nB=nB)


def _prep_relation(src, dst, n_src, n_dst, feat, rho):
    """Host-side sharding/packing for one relation.

    rho: estimated slot merge rate (pair-served edge fraction) used to pack
    tiles to near-multiple-of-128 slot counts.
    """
    src = np.asarray(src, np.int64)
    dst = np.asarray(dst, np.int64)
    deg_s = np.maximum(np.bincount(src, minlength=n_src), 1).astype(np.float64)
    deg_d_raw = np.bincount(dst, minlength=n_dst)
    deg_d = np.maximum(deg_d_raw, 1).astype(np.float64)
    rs_s = (1.0 / np.sqrt(deg_s)).astype(np.float32)
    rs_d = (1.0 / np.sqrt(deg_d)).astype(np.float32)
    u_d = np.sqrt(deg_d).astype(np.float32)  # ~= 1/rs_d

    feat_s = (np.asarray(feat, np.float32) * rs_s[:, None]).astype(np.float32)

    D = _rup(_cdiv(n_dst, NCORES), 128)  # dst rows per core (padded)
    ntiles = D // 128

    dst_ids = _balanced_assign(deg_d_raw.astype(np.int64), n_dst, ntiles, rho)
    slot_core = np.empty(n_dst, np.int64)
    slot_loc = np.empty(n_dst, np.int64)
    for c in range(NCORES):
        m = dst_ids[c] >= 0
        slot_core[dst_ids[c, m]] = c
        slot_loc[dst_ids[c, m]] = np.nonzero(m)[0]

    e_core = slot_core[dst]
    e_loc = slot_loc[dst]
    cores = []
    for c in range(NCORES):
        m = e_core == c
        pk = _pack_core(src[m], e_loc[m], n_src, ntiles)
        pk["dst_ids"] = dst_ids[c].copy()
        cores.append(pk)

    # rank-match: per core sort its tiles by slot count desc so tile index t
    # has similar (max-over-core) block counts
    for c in range(NCORES):
        pk = cores[c]
        perm = np.argsort(-pk["nslot"], kind="stable")
        pk["tiles"] = [pk["tiles"][t] for t in perm]
        pk["nslot"] = pk["nslot"][perm]
        pk["nB"] = pk["nB"][perm]
        pk["dst_ids"] = pk["dst_ids"].reshape(ntiles, 128)[perm].reshape(-1)

    nslot_all = np.stack([cores[c]["nslot"] for c in range(NCORES)])
    nB_all = np.stack([cores[c]["nB"] for c in range(NCORES)])
    btile = np.maximum(_cdiv(np.max(nslot_all, axis=0), BLK), 1)
    nbtile = np.minimum(_cdiv(np.max(nB_all, axis=0), BLK), btile)
    nrows = max(len(cores[c]["table_rows"]) for c in range(NCORES))

    return dict(cores=cores, btile=btile, nbtile=nbtile, nrows=nrows,
                nblk=int(btile.sum()), ntiles=ntiles, D=D,
                feat_s=feat_s, rs_d=rs_d, u_d=u_d, n_dst=n_dst)


def _build_host_data(inputs):
    rels = [
        # order matters: output rows are [node_out, inst_out, svc_out].
        # rho = measured pair-merge rate per relation on this graph
        _prep_relation(inputs["in_src"], inputs["in_dst"], INST_N, NODE_N,
                       inputs["instance_feat"], rho=0.49),
        _prep_relation(inputs["ni_src"], inputs["ni_dst"], NODE_N, INST_N,
                       inputs["node_feat"], rho=0.30),
        _prep_relation(inputs["sc_src"], inputs["sc_dst"], SVC_N, SVC_N,
                       inputs["svc_feat"], rho=0.41),
    ]
    Ws = [inputs["W_inst"], inputs["W_node"], inputs["W_svc"]]
    bs = [inputs["b_inst"], inputs["b_node"], inputs["b_svc"]]

    # per-relation gather chunk size minimizing relation-tail pad blocks
    # (pad descriptors are charged by the DMA model even when reg-trimmed)
    def _best_chunk(nblk):
        return min(range(15, 25),
                   key=lambda cc: (_rup(nblk, cc) - nblk, abs(cc - CHUNK)))

    chunks = [_best_chunk(r["nblk"]) for r in rels]
    nblk_pads = [_rup(r["nblk"], chunks[i]) for i, r in enumerate(rels)]
    nblk_tot = sum(nblk_pads)
    nidx_tot = nblk_tot * BLK
    ntile_tot = sum(r["ntiles"] for r in rels)

    W_cat = np.concatenate([np.asarray(w, np.float32) for w in Ws],
                           axis=1).astype(np.float16)
    b_row = np.concatenate([np.asarray(b, np.float32) for b in bs]
                           )[None, :].astype(np.float16)
    iota = np.tile(np.arange(128, dtype=np.float32), (128, 1)).astype(np.float16)

    in_maps = []
    for c in range(NCORES):
        gidx = np.full(nidx_tot, -1, np.int64)
        dlA = np.full(nidx_tot, -1.0, np.float32)
        dlB = np.full(nidx_tot, -1.0, np.float32)
        tbls = []
        rel_bases = np.r_[0, np.cumsum(nblk_pads)]
        for ri, r in enumerate(rels):
            off = int(rel_bases[ri])  # block offset in global stream
            pk = r["cores"][c]
            assert r["nrows"] < 32768, "row idx must fit int16"
            # staggered-duplicate pair rows: tbl[i] = [feat(i), feat(i+1)]
            fr = np.zeros((r["nrows"] + 1, HID), np.float16)
            tr = pk["table_rows"]
            fr[:len(tr)] = r["feat_s"][tr].astype(np.float16)
            tbl = np.concatenate([fr[:-1], fr[1:]], axis=1)
            tbls.append(np.ascontiguousarray(tbl))
            for t in range(r["ntiles"]):
                sp, da, db = pk["tiles"][t]
                n = len(sp)
                base = off * BLK
                gidx[base:base + n] = sp
                gidx[base + n: base + int(r["btile"][t]) * BLK] = 0
                dlA[base:base + n] = da
                dlB[base:base + n] = db
                off += int(r["btile"][t])
            # relation-tail pad blocks keep idx -1 (trimmed device-side)

        idx16 = np.ascontiguousarray(
            gidx.astype(np.int16).reshape(-1, 16).T)
        idx_sb = np.tile(idx16, (8, 1))                          # [128, nidx/16]
        dlA_sb = np.ascontiguousarray(dlA.reshape(nblk_tot, BLK).T)
        dlB_sb = np.ascontiguousarray(dlB.reshape(nblk_tot, BLK).T)

        rs_sb = np.zeros((128, ntile_tot), np.float32)
        u_sb = np.zeros((1, ntile_tot * 128), np.float32)
        t0 = 0
        for r in rels:
            ids = r["cores"][c]["dst_ids"]
            val_rs = np.zeros(r["D"], np.float32)
            val_u = np.zeros(r["D"], np.float32)
            m = ids >= 0
            val_rs[m] = r["rs_d"][ids[m]]
            val_u[m] = r["u_d"][ids[m]]
            rs_sb[:, t0:t0 + r["ntiles"]] = val_rs.reshape(r["ntiles"], 128).T
            u_sb[0, t0 * 128:(t0 + r["ntiles"]) * 128] = val_u
            t0 += r["ntiles"]
        u_sb = u_sb.astype(np.float16)

        in_maps.append({
            "tbl_in": tbls[0],
            "tbl_ni": tbls[1],
            "tbl_sc": tbls[2],
            "idx_sb": np.ascontiguousarray(idx_sb),
            "dlA_sb": dlA_sb,
            "dlB_sb": dlB_sb,
            "rs_sb": rs_sb,
            "u_sb": u_sb,
            "W_cat": np.ascontiguousarray(W_cat),
            "b_row": np.ascontiguousarray(b_row),
            "iota": np.ascontiguousarray(iota),
        })

    meta = dict(
        chunks=chunks,
        nblk_tot=nblk_tot, nidx_tot=nidx_tot, ntile_tot=ntile_tot,
        nrowss=[r["nrows"] for r in rels],
        btiles=[r["btile"].tolist() for r in rels],
        nbtiles=[r["nbtile"].tolist() for r in rels],
        ntiles=[r["ntiles"] for r in rels],
        Ds=[r["D"] for r in rels],
        n_dsts=[r["n_dst"] for r in rels],
        dst_ids=[[r["cores"][c]["dst_ids"] for c in range(NCORES)]
                 for r in rels],
    )
    return meta, in_maps


def _build_program(meta):
    import os

    import concourse.bacc as bacc
    import concourse.mybir as mybir
    import concourse.tile as tile

    dbg_max_tiles = int(os.environ.get("GNN_MAX_TILES", "0"))  # 0 = all

    gdt = mybir.dt.float16
    f16 = mybir.dt.float16
    f32 = mybir.dt.float32
    AF = mybir.ActivationFunctionType
    act_fn = AF.Lrelu if ACT_MODE == "lrelu" else AF.Relu

    nblk_tot, nidx_tot, ntile_tot = (meta["nblk_tot"], meta["nidx_tot"],
                                     meta["ntile_tot"])

    nc = bacc.Bacc("TRN2", target_bir_lowering=False, debug=False,
                   enable_asserts=False, num_devices=NCORES)

    tbl_d = [
        nc.dram_tensor("tbl_in", [meta["nrowss"][0], PAIR], gdt,
                       kind="ExternalInput"),
        nc.dram_tensor("tbl_ni", [meta["nrowss"][1], PAIR], gdt,
                       kind="ExternalInput"),
        nc.dram_tensor("tbl_sc", [meta["nrowss"][2], PAIR], gdt,
                       kind="ExternalInput"),
    ]
    idx_d = nc.dram_tensor("idx_sb", [128, nidx_tot // 16], mybir.dt.int16,
                           kind="ExternalInput")
    dlA_d = nc.dram_tensor("dlA_sb", [128, nblk_tot], f32, kind="ExternalInput")
    dlB_d = nc.dram_tensor("dlB_sb", [128, nblk_tot], f32, kind="ExternalInput")
    rs_d = nc.dram_tensor("rs_sb", [128, ntile_tot], f32, kind="ExternalInput")
    u_d = nc.dram_tensor("u_sb", [1, ntile_tot * 128], f16, kind="ExternalInput")
    W_d = nc.dram_tensor("W_cat", [128, 3 * HID], f16, kind="ExternalInput")
    b_d = nc.dram_tensor("b_row", [1, 3 * HID], f16, kind="ExternalInput")
    iota_d = nc.dram_tensor("iota", [128, 128], gdt, kind="ExternalInput")

    # p-major grouped layout: row g*128+p holds OUT_GRP tiles' rows for dst
    # slot p — out DMA descriptors are OUT_GRP*256B contiguous (no sub-512B
    # DMA bus penalty); host assemble untangles
    out_d = [
        nc.dram_tensor(n, [_cdiv(meta["ntiles"][i], OUT_GRP) * 128,
                           OUT_GRP * HID], f16, kind="ExternalOutput")
        for i, n in enumerate(["out_node", "out_inst", "out_svc"])
    ]

    with tile.TileContext(nc) as tc:
        with (
            tc.tile_pool(name="const", bufs=1) as const,
            tc.tile_pool(name="g", bufs=int(os.environ.get("GNN_GBUFS", "7"))) as gpool,
            tc.tile_pool(name="st", bufs=int(os.environ.get("GNN_STBUFS", "48"))) as stpool,
            tc.tile_pool(name="evac", bufs=int(os.environ.get("GNN_EVBUFS", "6"))) as evac,
            tc.tile_pool(name="osb", bufs=int(os.environ.get("GNN_OBUFS", "6"))) as opool,
            tc.tile_pool(name="psA", bufs=5, space="PSUM") as psA,
            tc.tile_pool(name="psO", bufs=3, space="PSUM") as psO,
        ):
            rel_nblks = [sum(meta["btiles"][r]) for r in range(3)]
            rel_base = [0, 0, 0]  # global block base per relation
            chs = meta["chunks"]
            for r in range(1, 3):
                rel_base[r] = rel_base[r - 1] + _rup(rel_nblks[r - 1],
                                                     chs[r - 1])
            tg_base = [0, meta["ntiles"][0],
                       meta["ntiles"][0] + meta["ntiles"][1]]

            # first gathers depend only on the leading idx slices + dl/iota:
            # load those first so the gather stream starts ASAP (HWDGE is
            # FIFO per issuing engine)
            idx_t = const.tile([128, nidx_tot // 16], mybir.dt.int16)
            heads = []
            for r in range(3):
                h0 = rel_base[r] * BLK // 16
                h1 = min(h0 + 2 * chs[r] * BLK // 16, nidx_tot // 16)
                heads.append((h0, h1))
                nc.sync.dma_start(idx_t[:, h0:h1], idx_d.ap()[:, h0:h1])
            dlA_t = const.tile([128, nblk_tot], f32)
            nc.sync.dma_start(dlA_t[:], dlA_d.ap())
            dlB_t = const.tile([128, nblk_tot], f32)
            nc.sync.dma_start(dlB_t[:], dlB_d.ap())
            iota_t = const.tile([128, 128], gdt)
            nc.sync.dma_start(iota_t[:], iota_d.ap())
            W_t = const.tile([128, 3 * HID], f16)
            nc.sync.dma_start(W_t[:], W_d.ap())
            b_t = const.tile([1, 3 * HID], f16)
            nc.sync.dma_start(b_t[:], b_d.ap())
            u_t = const.tile([1, ntile_tot * 128], f16)
            nc.sync.dma_start(u_t[:], u_d.ap())
            rs_t = const.tile([128, ntile_tot], f32)
            nc.sync.dma_start(rs_t[:], rs_d.ap())
            for r in range(3):
                h1 = heads[r][1]
                end = rel_base[r] * BLK // 16 + \
                    _rup(rel_nblks[r], chs[r]) * BLK // 16
                if h1 < end:
                    nc.sync.dma_start(idx_t[:, h1:end], idx_d.ap()[:, h1:end])

            g_tiles = {}   # (rel, rel-local chunk) -> gather tile

            def issue_gather(rel, lci):
                ch = chs[rel]
                gt = gpool.tile([128, ch, PAIR], gdt, tag="g")
                nidx = ch * BLK
                h0 = rel_base[rel] * BLK // 16 + lci * (nidx // 16)
                # trailing -1 idxs (relation-tail pads) are skipped; trim reg
                real_blocks = max(0, min(ch, rel_nblks[rel] - lci * ch))
                nc.gpsimd.dma_gather(
                    out_ap=gt[:],
                    in_ap=tbl_d[rel].ap(),
                    idxs_ap=idx_t[:, h0:h0 + nidx // 16],
                    num_idxs=nidx,
                    num_idxs_reg=max(BLK, real_blocks * BLK),
                    elem_size=PAIR,
                    single_packet=False,
                )
                g_tiles[(rel, lci)] = gt

            # relation-interleaved tile schedule: spread the epilogue-heavy
            # relation (ni: many tiles, few blocks) evenly across the
            # gather-heavy one (sc) so no engine's work bunches up
            sched = []
            scale = [0.93, 0.95, 1.0]  # in/ni finish early; sc's last blocks
            for r in range(3):         # keep gathers flowing during drain
                btile = meta["btiles"][r]
                o = 0
                for t in range(meta["ntiles"][r]):
                    if dbg_max_tiles and t >= dbg_max_tiles:
                        break
                    # key on end-fraction: the block-heavy relation's last
                    # tile keeps gathers flowing while light tiles' epilogues
                    # drain, shrinking the no-DMA tail
                    sched.append((scale[r] * (o + btile[t])
                                  / max(1, rel_nblks[r]), r, t))
                    o += btile[t]
            sched.sort()

            rel_blk = [0, 0, 0]       # relation-local block cursor
            osb_state = [None, None, None]

            for _, rel, t in sched:
                ntiles = meta["ntiles"][rel]
                nb = meta["btiles"][rel][t]
                nbB = meta["nbtiles"][rel][t]
                agg = psA.tile([128, 128], f32, tag="agg")
                for b in range(nb):
                    lb = rel_blk[rel]
                    lci, cj = divmod(lb, chs[rel])
                    if cj == 0:
                        issue_gather(rel, lci)
                    blk = rel_base[rel] + lb  # global dl column
                    has_b = b < nbB
                    stA = stpool.tile([128, 128], gdt, tag="stA")
                    nc.vector.tensor_scalar(
                        stA[:], iota_t[:], dlA_t[:, blk:blk + 1], None,
                        mybir.AluOpType.is_equal)
                    last = b == nb - 1
                    nc.tensor.matmul(
                        agg[:], g_tiles[(rel, lci)][:, cj, 0:HID], stA[:],
                        start=(b == 0), stop=(last and not has_b))
                    if has_b:
                        stB = stpool.tile([128, 128], gdt, tag="stB")
                        nc.vector.tensor_scalar(
                            stB[:], iota_t[:], dlB_t[:, blk:blk + 1], None,
                            mybir.AluOpType.is_equal)
                        nc.tensor.matmul(
                            agg[:], g_tiles[(rel, lci)][:, cj, HID:PAIR], stB[:],
                            start=False, stop=last)
                    rel_blk[rel] += 1
                tglob = tg_base[rel] + t
                aggsb = evac.tile([128, 128], f16, tag="evac")
                nc.scalar.copy(aggsb[:], agg[:])
                po = psO.tile([128, 128], f32, tag="po")
                nc.tensor.matmul(
                    po[:], u_t[:, tglob * 128:(tglob + 1) * 128],
                    b_t[:, rel * HID:(rel + 1) * HID],
                    start=True, stop=False, skip_group_check=True)
                nc.tensor.matmul(
                    po[:], aggsb[:], W_t[:, rel * HID:(rel + 1) * HID],
                    start=False, stop=True, skip_group_check=True)
                oj = t % OUT_GRP
                if oj == 0:
                    osb_state[rel] = (
                        opool.tile([128, OUT_GRP, 128], f16, tag="osb",
                                   name="osb"), t)
                osb, osb_t0 = osb_state[rel]
                nc.scalar.activation(
                    osb[:, oj, :], po[:], act_fn,
                    bias=0.0, scale=rs_t[:, tglob:tglob + 1], alpha=0.01)
                if oj == OUT_GRP - 1 or t == ntiles - 1:
                    cnt = t - osb_t0 + 1
                    g = osb_t0 // OUT_GRP
                    dst = out_d[rel].ap()[g * 128:(g + 1) * 128, :cnt * HID]
                    nc.sync.dma_start(
                        dst.rearrange("p (j k) -> p j k", k=HID),
                        osb[:, :cnt, :])

    nc.compile()
    return nc


def _run(nc, in_maps, trace=False, **kw):
    from concourse import bass_utils
    res = bass_utils.run_bass_kernel_spmd(
        nc, in_maps, core_ids=list(range(NCORES)), trace=trace, **kw)
    return res


def _assemble(results, meta):
    out = np.empty((NODE_N + INST_N + SVC_N, HID), np.float32)
    offs = [0, NODE_N, NODE_N + INST_N]
    names = ["out_node", "out_inst", "out_svc"]
    for rel in range(3):
        nt = meta["ntiles"][rel]
        ngrp = _cdiv(nt, OUT_GRP)
        for c in range(NCORES):
            ids = meta["dst_ids"][rel][c]
            m = ids >= 0
            arr = np.asarray(results[c][names[rel]], np.float32)
            rows = arr.reshape(ngrp, 128, OUT_GRP, HID).transpose(
                0, 2, 1, 3).reshape(ngrp * OUT_GRP * 128, HID)[:nt * 128]
            out[offs[rel] + ids[m]] = rows[m]
    return out


def kernel(**inputs):
    import hashlib
    key = "prog"
    h = hashlib.sha1()
    for k in ("sc_src", "sc_dst", "in_src", "in_dst", "ni_src", "ni_dst"):
        h.update(np.ascontiguousarray(np.asarray(inputs[k], np.int32)).tobytes())
    sig = h.hexdigest()
    meta, in_maps = _build_host_data(inputs)
    if key in _cache and _cache[key][0] == sig:
        _, nc, _ = _cache[key]
    else:
        nc = _build_program(meta)
        _cache[key] = (sig, nc, meta)
    res = _run(nc, in_maps)
    return _assemble(res.results, meta)


# revision 60
# speedup vs baseline: 1.0548x; 1.0094x over previous
"""Trainium2 Bass kernel for a heterogeneous GraphConv layer (3 relations).

out = concat([leaky(GC(inst_feat, W_inst, in_*)),     # -> node   (10000)
              leaky(GC(node_feat, W_node, ni_*)),     # -> inst   (100000)
              leaky(GC(svc_feat,  W_svc,  sc_*))])    # -> svc    (20000)

GC(f, W, src, dst) = rsqrt(deg_d) * segsum_dst((rsqrt(deg_s)*f)[src]) @ W + b
(aggregation commutes with the dense @W, so we gather *raw scaled features*
and apply W once per destination tile).

Strategy: destination-sharded across 8 NeuronCores, with host-balanced
dst->(core,tile,slot) assignment (tile edge sums packed to multiples of 128
and rank-matched across cores so the shared program's per-tile block counts
stay near the mean).

DMA-gather cost on TRN2 is per-descriptor: a 512B descriptor costs the same
as 256B (sub-512B transfers pay a 2x bus penalty), so the gather tables are
laid out as PAIR rows [2*128] fp16 = 512B: one descriptor fetches two
feature rows. The per-core table is ordered by first use so the two edges
that introduce a pair of new sources in the same dst tile share one
descriptor ("paired slots"); repeat edges use one half and the other half
rides free (and occasionally serves a second edge whose source lands on the
pair sibling). Per slot there are two one-hot channels dlA/dlB (-1 = unused)
selecting the dst column for the low/high half row.

Device work per 128-slot block: dma_gather 128 pair rows -> [128e, 256f],
DVE tensor_scalar builds one-hot stA[e,d]=(iota==dlA) (and stB for the
pair-slot prefix blocks only), PE accumulates aggT[f,d] += A.T@stA (+B.T@stB)
in PSUM. Per dst tile: PSUM out = u (x) b (rank-1 K=1 fp32 matmul preloading
bias) + aggT.T @ W (fp16), then one ScalarE Lrelu(out * rsqrt_deg_d) and a
grouped fp16 DMA to the output rows.
"""

import os as _os

import numpy as np

SVC_N, INST_N, NODE_N, HID = 20000, 100000, 10000, 128
NCORES = 8
BLK = 128           # slots per one-hot matmul block
# blocks per dma_gather instruction. NOTE: needs single_packet=False — with
# single_packet=True the whole stream coalesces into one DMA packet, which
# caps at 64 descriptors/engine (num_idxs <= 1024); beyond that the exec
# unit faults (NRT_EXEC_UNIT_UNRECOVERABLE).
CHUNK = int(_os.environ.get("GNN_CHUNK", "16"))
GDT = "fp16"
ACT_MODE = "lrelu"  # "lrelu" (HW leaky relu) | "relu" (sim debug)
PAIR = 2 * HID      # table row = pair of feature rows (512B fp16)
# dst tiles batched per epilogue staging buffer / out DMA
OUT_GRP = int(_os.environ.get("GNN_OUT_GRP", "8"))

_cache = {}


def _cdiv(a, b):
    return (a + b - 1) // b


def _rup(a, b):
    return _cdiv(a, b) * b


def _balanced_assign(deg, n_dst, ntiles, rho):
    """Assign dst nodes to (core, tile, slot) packing per-tile SLOT counts
    (estimated as edges*(1-rho), rho = relation merge rate) just under
    multiples of 128 so per-tile block counts carry minimal ceil padding.

    Returns dst_ids[NCORES, ntiles*128] int64 (-1 = pad slot): the global dst
    node stored at each (core, tile, slot).
    """
    order = np.argsort(-deg, kind="stable")
    # snake over cores -> per-core totals equal to within one max-degree
    core_of = np.empty(n_dst, np.int64)
    snake = np.concatenate([np.arange(NCORES), np.arange(NCORES)[::-1]])
    core_of[order] = snake[np.arange(n_dst) % (2 * NCORES)]

    dst_ids = np.full((NCORES, ntiles * 128), -1, np.int64)
    for c in range(NCORES):
        ids = order[core_of[order] == c]  # degree-descending
        w = deg[ids]
        total = int(w.sum())
        # uniform edge-weight target per bin (multiple of 128; rho reserved
        # for a future slot-aware quota scheme — measured merge-rate noise
        # and the max-over-core coupling made per-bin quotas regress)
        target = np.full(ntiles, _cdiv(total, ntiles * 128) * 128, np.int64)
        binw = np.zeros(ntiles, np.int64)
        binn = np.zeros(ntiles, np.int64)
        bins = [[] for _ in range(ntiles)]
        # greedy: place each dst (deg desc) in the fullest bin it still fits
        # (by weight target and 128-slot cap); else least-filled open bin
        open_bins = list(range(ntiles))
        for i, d in zip(ids, w):
            best, bestw = -1, -1
            for t in open_bins:
                if binw[t] + d <= target[t] and binw[t] > bestw:
                    best, bestw = t, binw[t]
            if best < 0:
                best = min(open_bins, key=lambda x: binw[x])
            t = best
            bins[t].append(i)
            binw[t] += d
            binn[t] += 1
            if binn[t] >= 128:
                open_bins.remove(t)
        for t in range(ntiles):
            ids_t = bins[t]
            dst_ids[c, t * 128: t * 128 + len(ids_t)] = ids_t
    return dst_ids


def _pack_core(es, ed, n_src, ntiles):
    """Pair-slot packing for one (core, relation).

    es: edge source node ids; ed: edge dst slot (tile*128 + dst_local).

    The gather table holds one 512B row per USED source: [feat(r), feat(r+1)]
    (staggered duplicate), so descriptor idx r serves edge(s) on table row r
    via channel A and optionally a second edge on row r+1 via channel B.
    Rows are ordered by their tile-usage lists (lexsort) so edges of the same
    tile sit on adjacent rows and merge into shared slots.

    Returns dict with: table_rows (src id per table row), per-tile slot
    arrays (row idx, dlA, dlB), nslot[t], nB[t].
    """
    KEYLEN = 6
    tile_of = (ed >> 7).astype(np.int64)
    dl = (ed & 127).astype(np.int64)

    rows_used = np.unique(es)
    nrows = len(rows_used)
    rid_of = np.full(n_src, -1, np.int64)
    rid_of[rows_used] = np.arange(nrows)

    # tile-usage key per row: first KEYLEN tiles (sorted), padded
    pt = np.unique(np.stack([rid_of[es], tile_of], axis=1), axis=0)
    grp_new = np.r_[True, pt[1:, 0] != pt[:-1, 0]]
    idx = np.arange(len(pt))
    j = idx - np.maximum.accumulate(np.where(grp_new, idx, 0))
    keymat = np.full((nrows, KEYLEN), 32767, np.int64)
    m = j < KEYLEN
    keymat[pt[m, 0], j[m]] = pt[m, 1]
    order = np.lexsort(keymat.T[::-1])
    table_rows = rows_used[order]          # src id at each table position
    pos_of = np.full(n_src, -1, np.int64)
    pos_of[table_rows] = np.arange(nrows)

    r = pos_of[es]
    # per tile: sort edges by table position; pair edges on consecutive
    # positions (runs split on gaps/duplicates, paired (0,1)(2,3)... in-run)
    o = np.lexsort((r, tile_of))
    kt, kr, kdl = tile_of[o], r[o], dl[o]
    brk = np.r_[True, (kt[1:] != kt[:-1]) | (kr[1:] != kr[:-1] + 1)]
    idx = np.arange(len(kt))
    k_in_run = idx - np.maximum.accumulate(np.where(brk, idx, 0))
    run_id = np.cumsum(brk) - 1
    half = k_in_run & 1
    skey = np.stack([run_id, k_in_run >> 1], axis=1)
    uslot, inv = np.unique(skey, axis=0, return_inverse=True)
    ns = len(uslot)
    s_tile = np.zeros(ns, np.int64)
    s_row = np.zeros(ns, np.int64)
    dlA = np.full(ns, -1, np.int64)
    dlB = np.full(ns, -1, np.int64)
    m0 = half == 0
    s_tile[inv[m0]] = kt[m0]
    s_row[inv[m0]] = kr[m0]
    dlA[inv[m0]] = kdl[m0]
    dlB[inv[~m0]] = kdl[~m0]
    has_b = dlB >= 0

    # order slots per tile: B-present first (so stB/matmul-B run only on a
    # prefix of blocks), then by row for gather locality
    so = np.lexsort((s_row, ~has_b, s_tile))
    s_tile, s_row, dlA, dlB, has_b = (
        s_tile[so], s_row[so], dlA[so], dlB[so], has_b[so])

    nslot = np.bincount(s_tile, minlength=ntiles)
    nB = np.bincount(s_tile[has_b], minlength=ntiles)
    tstart = np.r_[0, np.cumsum(nslot)]
    tiles = []
    for t in range(ntiles):
        sl = slice(tstart[t], tstart[t + 1])
        tiles.append((s_row[sl], dlA[sl], dlB[sl]))
    return dict(table_rows=table_rows, tiles=tiles, nslot=nslot, nB=nB)


def _prep_relation(src, dst, n_src, n_dst, feat, rho):
    """Host-side sharding/packing for one relation.

    rho: estimated slot merge rate (pair-served edge fraction) used to pack
    tiles to near-multiple-of-128 slot counts.
    """
    src = np.asarray(src, np.int64)
    dst = np.asarray(dst, np.int64)
    deg_s = np.maximum(np.bincount(src, minlength=n_src), 1).astype(np.float64)
    deg_d_raw = np.bincount(dst, minlength=n_dst)
    deg_d = np.maximum(deg_d_raw, 1).astype(np.float64)
    rs_s = (1.0 / np.sqrt(deg_s)).astype(np.float32)
    rs_d = (1.0 / np.sqrt(deg_d)).astype(np.float32)
    u_d = np.sqrt(deg_d).astype(np.float32)  # ~= 1/rs_d

    feat_s = (np.asarray(feat, np.float32) * rs_s[:, None]).astype(np.float32)

    D = _rup(_cdiv(n_dst, NCORES), 128)  # dst rows per core (padded)
    ntiles = D // 128

    dst_ids = _balanced_assign(deg_d_raw.astype(np.int64), n_dst, ntiles, rho)
    slot_core = np.empty(n_dst, np.int64)
    slot_loc = np.empty(n_dst, np.int64)
    for c in range(NCORES):
        m = dst_ids[c] >= 0
        slot_core[dst_ids[c, m]] = c
        slot_loc[dst_ids[c, m]] = np.nonzero(m)[0]

    e_core = slot_core[dst]
    e_loc = slot_loc[dst]
    cores = []
    for c in range(NCORES):
        m = e_core == c
        pk = _pack_core(src[m], e_loc[m], n_src, ntiles)
        pk["dst_ids"] = dst_ids[c].copy()
        cores.append(pk)

    # rank-match: per core sort its tiles by slot count desc so tile index t
    # has similar (max-over-core) block counts
    for c in range(NCORES):
        pk = cores[c]
        perm = np.argsort(-pk["nslot"], kind="stable")
        pk["tiles"] = [pk["tiles"][t] for t in perm]
        pk["nslot"] = pk["nslot"][perm]
        pk["nB"] = pk["nB"][perm]
        pk["dst_ids"] = pk["dst_ids"].reshape(ntiles, 128)[perm].reshape(-1)

    nslot_all = np.stack([cores[c]["nslot"] for c in range(NCORES)])
    nB_all = np.stack([cores[c]["nB"] for c in range(NCORES)])
    btile = np.maximum(_cdiv(np.max(nslot_all, axis=0), BLK), 1)
    nbtile = np.minimum(_cdiv(np.max(nB_all, axis=0), BLK), btile)
    nrows = max(len(cores[c]["table_rows"]) for c in range(NCORES))

    return dict(cores=cores, btile=btile, nbtile=nbtile, nrows=nrows,
                nblk=int(btile.sum()), ntiles=ntiles, D=D,
                feat_s=feat_s, rs_d=rs_d, u_d=u_d, n_dst=n_dst)


def _build_host_data(inputs):
    rels = [
        # order matters: output rows are [node_out, inst_out, svc_out].
        # rho = measured pair-merge rate per relation on this graph
        _prep_relation(inputs["in_src"], inputs["in_dst"], INST_N, NODE_N,
                       inputs["instance_feat"], rho=0.49),
        _prep_relation(inputs["ni_src"], inputs["ni_dst"], NODE_N, INST_N,
                       inputs["node_feat"], rho=0.30),
        _prep_relation(inputs["sc_src"], inputs["sc_dst"], SVC_N, SVC_N,
                       inputs["svc_feat"], rho=0.41),
    ]
    Ws = [inputs["W_inst"], inputs["W_node"], inputs["W_svc"]]
    bs = [inputs["b_inst"], inputs["b_node"], inputs["b_svc"]]

    # per-relation gather chunk size minimizing relation-tail pad blocks
    # (pad descriptors are charged by the DMA model even when reg-trimmed)
    def _best_chunk(nblk):
        return min(range(15, 25),
                   key=lambda cc: (_rup(nblk, cc) - nblk, abs(cc - CHUNK)))

    chunks = [_best_chunk(r["nblk"]) for r in rels]
    nblk_pads = [_rup(r["nblk"], chunks[i]) for i, r in enumerate(rels)]
    nblk_tot = sum(nblk_pads)
    nidx_tot = nblk_tot * BLK
    ntile_tot = sum(r["ntiles"] for r in rels)

    W_cat = np.concatenate([np.asarray(w, np.float32) for w in Ws],
                           axis=1).astype(np.float16)
    b_row = np.concatenate([np.asarray(b, np.float32) for b in bs]
                           )[None, :].astype(np.float16)
    iota = np.tile(np.arange(128, dtype=np.float32), (128, 1)).astype(np.float16)

    in_maps = []
    for c in range(NCORES):
        gidx = np.full(nidx_tot, -1, np.int64)
        dlA = np.full(nidx_tot, -1.0, np.float32)
        dlB = np.full(nidx_tot, -1.0, np.float32)
        tbls = []
        rel_bases = np.r_[0, np.cumsum(nblk_pads)]
        for ri, r in enumerate(rels):
            off = int(rel_bases[ri])  # block offset in global stream
            pk = r["cores"][c]
            assert r["nrows"] < 32768, "row idx must fit int16"
            # staggered-duplicate pair rows: tbl[i] = [feat(i), feat(i+1)]
            fr = np.zeros((r["nrows"] + 1, HID), np.float16)
            tr = pk["table_rows"]
            fr[:len(tr)] = r["feat_s"][tr].astype(np.float16)
            tbl = np.concatenate([fr[:-1], fr[1:]], axis=1)
            tbls.append(np.ascontiguousarray(tbl))
            for t in range(r["ntiles"]):
                sp, da, db = pk["tiles"][t]
                n = len(sp)
                base = off * BLK
                gidx[base:base + n] = sp
                gidx[base + n: base + int(r["btile"][t]) * BLK] = 0
                dlA[base:base + n] = da
                dlB[base:base + n] = db
                off += int(r["btile"][t])
            # relation-tail pad blocks keep idx -1 (trimmed device-side)

        idx16 = np.ascontiguousarray(
            gidx.astype(np.int16).reshape(-1, 16).T)
        idx_sb = np.tile(idx16, (8, 1))                          # [128, nidx/16]
        dlA_sb = np.ascontiguousarray(dlA.reshape(nblk_tot, BLK).T)
        dlB_sb = np.ascontiguousarray(dlB.reshape(nblk_tot, BLK).T)

        rs_sb = np.zeros((128, ntile_tot), np.float32)
        u_sb = np.zeros((1, ntile_tot * 128), np.float32)
        t0 = 0
        for r in rels:
            ids = r["cores"][c]["dst_ids"]
            val_rs = np.zeros(r["D"], np.float32)
            val_u = np.zeros(r["D"], np.float32)
            m = ids >= 0
            val_rs[m] = r["rs_d"][ids[m]]
            val_u[m] = r["u_d"][ids[m]]
            rs_sb[:, t0:t0 + r["ntiles"]] = val_rs.reshape(r["ntiles"], 128).T
            u_sb[0, t0 * 128:(t0 + r["ntiles"]) * 128] = val_u
            t0 += r["ntiles"]
        u_sb = u_sb.astype(np.float16)

        in_maps.append({
            "tbl_in": tbls[0],
            "tbl_ni": tbls[1],
            "tbl_sc": tbls[2],
            "idx_sb": np.ascontiguousarray(idx_sb),
            "dlA_sb": dlA_sb,
            "dlB_sb": dlB_sb,
            "rs_sb": rs_sb,
            "u_sb": u_sb,
            "W_cat": np.ascontiguousarray(W_cat),
            "b_row": np.ascontiguousarray(b_row),
            "iota": np.ascontiguousarray(iota),
        })

    meta = dict(
        chunks=chunks,
        nblk_tot=nblk_tot, nidx_tot=nidx_tot, ntile_tot=ntile_tot,
        nrowss=[r["nrows"] for r in rels],
        btiles=[r["btile"].tolist() for r in rels],
        nbtiles=[r["nbtile"].tolist() for r in rels],
        ntiles=[r["ntiles"] for r in rels],
        Ds=[r["D"] for r in rels],
        n_dsts=[r["n_dst"] for r in rels],
        dst_ids=[[r["cores"][c]["dst_ids"] for c in range(NCORES)]
                 for r in rels],
    )
    return meta, in_maps


def _build_program(meta):
    import os

    import concourse.bacc as bacc
    import concourse.mybir as mybir
    import concourse.tile as tile

    dbg_max_tiles = int(os.environ.get("GNN_MAX_TILES", "0"))  # 0 = all

    gdt = mybir.dt.float16
    f16 = mybir.dt.float16
    f32 = mybir.dt.float32
    AF = mybir.ActivationFunctionType
    act_fn = AF.Lrelu if ACT_MODE == "lrelu" else AF.Relu

    nblk_tot, nidx_tot, ntile_tot = (meta["nblk_tot"], meta["nidx_tot"],
                                     meta["ntile_tot"])

    nc = bacc.Bacc("TRN2", target_bir_lowering=False, debug=False,
                   enable_asserts=False, num_devices=NCORES)

    tbl_d = [
        nc.dram_tensor("tbl_in", [meta["nrowss"][0], PAIR], gdt,
                       kind="ExternalInput"),
        nc.dram_tensor("tbl_ni", [meta["nrowss"][1], PAIR], gdt,
                       kind="ExternalInput"),
        nc.dram_tensor("tbl_sc", [meta["nrowss"][2], PAIR], gdt,
                       kind="ExternalInput"),
    ]
    idx_d = nc.dram_tensor("idx_sb", [128, nidx_tot // 16], mybir.dt.int16,
                           kind="ExternalInput")
    dlA_d = nc.dram_tensor("dlA_sb", [128, nblk_tot], f32, kind="ExternalInput")
    dlB_d = nc.dram_tensor("dlB_sb", [128, nblk_tot], f32, kind="ExternalInput")
    rs_d = nc.dram_tensor("rs_sb", [128, ntile_tot], f32, kind="ExternalInput")
    u_d = nc.dram_tensor("u_sb", [1, ntile_tot * 128], f16, kind="ExternalInput")
    W_d = nc.dram_tensor("W_cat", [128, 3 * HID], f16, kind="ExternalInput")
    b_d = nc.dram_tensor("b_row", [1, 3 * HID], f16, kind="ExternalInput")
    iota_d = nc.dram_tensor("iota", [128, 128], gdt, kind="ExternalInput")

    # p-major grouped layout: row g*128+p holds OUT_GRP tiles' rows for dst
    # slot p — out DMA descriptors are OUT_GRP*256B contiguous (no sub-512B
    # DMA bus penalty); host assemble untangles
    out_d = [
        nc.dram_tensor(n, [_cdiv(meta["ntiles"][i], OUT_GRP) * 128,
                           OUT_GRP * HID], f16, kind="ExternalOutput")
        for i, n in enumerate(["out_node", "out_inst", "out_svc"])
    ]

    with tile.TileContext(nc) as tc:
        with (
            tc.tile_pool(name="const", bufs=1) as const,
            tc.tile_pool(name="g", bufs=int(os.environ.get("GNN_GBUFS", "7"))) as gpool,
            tc.tile_pool(name="st", bufs=int(os.environ.get("GNN_STBUFS", "48"))) as stpool,
            tc.tile_pool(name="evac", bufs=int(os.environ.get("GNN_EVBUFS", "6"))) as evac,
            tc.tile_pool(name="osb", bufs=int(os.environ.get("GNN_OBUFS", "6"))) as opool,
            tc.tile_pool(name="psA", bufs=5, space="PSUM") as psA,
            tc.tile_pool(name="psO", bufs=3, space="PSUM") as psO,
        ):
            rel_nblks = [sum(meta["btiles"][r]) for r in range(3)]
            rel_base = [0, 0, 0]  # global block base per relation
            chs = meta["chunks"]
            for r in range(1, 3):
                rel_base[r] = rel_base[r - 1] + _rup(rel_nblks[r - 1],
                                                     chs[r - 1])
            tg_base = [0, meta["ntiles"][0],
                       meta["ntiles"][0] + meta["ntiles"][1]]

            # first gathers depend only on the leading idx slices + dl/iota:
            # load those first so the gather stream starts ASAP (HWDGE is
            # FIFO per issuing engine)
            idx_t = const.tile([128, nidx_tot // 16], mybir.dt.int16)
            heads = []
            for r in range(3):
                h0 = rel_base[r] * BLK // 16
                h1 = min(h0 + 2 * chs[r] * BLK // 16, nidx_tot // 16)
                heads.append((h0, h1))
                nc.sync.dma_start(idx_t[:, h0:h1], idx_d.ap()[:, h0:h1])
            dlA_t = const.tile([128, nblk_tot], f32)
            nc.sync.dma_start(dlA_t[:], dlA_d.ap())
            dlB_t = const.tile([128, nblk_tot], f32)
            nc.sync.dma_start(dlB_t[:], dlB_d.ap())
            iota_t = const.tile([128, 128], gdt)
            nc.sync.dma_start(iota_t[:], iota_d.ap())
            W_t = const.tile([128, 3 * HID], f16)
            nc.sync.dma_start(W_t[:], W_d.ap())
            b_t = const.tile([1, 3 * HID], f16)
            nc.sync.dma_start(b_t[:], b_d.ap())
            u_t = const.tile([1, ntile_tot * 128], f16)
            nc.sync.dma_start(u_t[:], u_d.ap())
            rs_t = const.tile([128, ntile_tot], f32)
            nc.sync.dma_start(rs_t[:], rs_d.ap())
            for r in range(3):
                h1 = heads[r][1]
                end = rel_base[r] * BLK // 16 + \
                    _rup(rel_nblks[r], chs[r]) * BLK // 16
                if h1 < end:
                    nc.sync.dma_start(idx_t[:, h1:end], idx_d.ap()[:, h1:end])

            g_tiles = {}   # (rel, rel-local chunk) -> gather tile

            def issue_gather(rel, lci):
                ch = chs[rel]
                gt = gpool.tile([128, ch, PAIR], gdt, tag="g")
                nidx = ch * BLK
                h0 = rel_base[rel] * BLK // 16 + lci * (nidx // 16)
                # trailing -1 idxs (relation-tail pads) are skipped; trim reg
                real_blocks = max(0, min(ch, rel_nblks[rel] - lci * ch))
                nc.gpsimd.dma_gather(
                    out_ap=gt[:],
                    in_ap=tbl_d[rel].ap(),
                    idxs_ap=idx_t[:, h0:h0 + nidx // 16],
                    num_idxs=nidx,
                    num_idxs_reg=max(BLK, real_blocks * BLK),
                    elem_size=PAIR,
                    single_packet=False,
                )
                g_tiles[(rel, lci)] = gt

            # relation-interleaved tile schedule: spread the epilogue-heavy
            # relation (ni: many tiles, few blocks) evenly across the
            # gather-heavy one (sc) so no engine's work bunches up
            sched = []
            scale = [0.92, 0.95, 1.0]  # in/ni finish early; sc's last blocks
            for r in range(3):         # keep gathers flowing during drain
                btile = meta["btiles"][r]
                o = 0
                for t in range(meta["ntiles"][r]):
                    if dbg_max_tiles and t >= dbg_max_tiles:
                        break
                    # key on end-fraction: the block-heavy relation's last
                    # tile keeps gathers flowing while light tiles' epilogues
                    # drain, shrinking the no-DMA tail
                    sched.append((scale[r] * (o + btile[t])
                                  / max(1, rel_nblks[r]), r, t))
                    o += btile[t]
            sched.sort()

            rel_blk = [0, 0, 0]       # relation-local block cursor
            osb_state = [None, None, None]

            for _, rel, t in sched:
                ntiles = meta["ntiles"][rel]
                nb = meta["btiles"][rel][t]
                nbB = meta["nbtiles"][rel][t]
                agg = psA.tile([128, 128], f32, tag="agg")
                for b in range(nb):
                    lb = rel_blk[rel]
                    lci, cj = divmod(lb, chs[rel])
                    if cj == 0:
                        issue_gather(rel, lci)
                    blk = rel_base[rel] + lb  # global dl column
                    has_b = b < nbB
                    stA = stpool.tile([128, 128], gdt, tag="stA")
                    nc.vector.tensor_scalar(
                        stA[:], iota_t[:], dlA_t[:, blk:blk + 1], None,
                        mybir.AluOpType.is_equal)
                    last = b == nb - 1
                    nc.tensor.matmul(
                        agg[:], g_tiles[(rel, lci)][:, cj, 0:HID], stA[:],
                        start=(b == 0), stop=(last and not has_b))
                    if has_b:
                        stB = stpool.tile([128, 128], gdt, tag="stB")
                        nc.vector.tensor_scalar(
                            stB[:], iota_t[:], dlB_t[:, blk:blk + 1], None,
                            mybir.AluOpType.is_equal)
                        nc.tensor.matmul(
                            agg[:], g_tiles[(rel, lci)][:, cj, HID:PAIR], stB[:],
                            start=False, stop=last)
                    rel_blk[rel] += 1
                tglob = tg_base[rel] + t
                aggsb = evac.tile([128, 128], f16, tag="evac")
                nc.scalar.copy(aggsb[:], agg[:])
                po = psO.tile([128, 128], f32, tag="po")
                nc.tensor.matmul(
                    po[:], u_t[:, tglob * 128:(tglob + 1) * 128],
                    b_t[:, rel * HID:(rel + 1) * HID],
                    start=True, stop=False, skip_group_check=True)
                nc.tensor.matmul(
                    po[:], aggsb[:], W_t[:, rel * HID:(rel + 1) * HID],
                    start=False, stop=True, skip_group_check=True)
                oj = t % OUT_GRP
                if oj == 0:
                    osb_state[rel] = (
                        opool.tile([128, OUT_GRP, 128], f16, tag="osb",
                                   name="osb"), t)
                osb, osb_t0 = osb_state[rel]
                nc.scalar.activation(
                    osb[:, oj, :], po[:], act_fn,
                    bias=0.0, scale=rs_t[:, tglob:tglob + 1], alpha=0.01)
                if oj == OUT_GRP - 1 or t == ntiles - 1:
                    cnt = t - osb_t0 + 1
                    g = osb_t0 // OUT_GRP
                    dst = out_d[rel].ap()[g * 128:(g + 1) * 128, :cnt * HID]
                    nc.sync.dma_start(
                        dst.rearrange("p (j k) -> p j k", k=HID),
                        osb[:, :cnt, :])

    nc.compile()
    return nc


def _run(nc, in_maps, trace=False, **kw):
    from concourse import bass_utils
    res = bass_utils.run_bass_kernel_spmd(
        nc, in_maps, core_ids=list(range(NCORES)), trace=trace, **kw)
    return res


def _assemble(results, meta):
    out = np.empty((NODE_N + INST_N + SVC_N, HID), np.float32)
    offs = [0, NODE_N, NODE_N + INST_N]
    names = ["out_node", "out_inst", "out_svc"]
    for rel in range(3):
        nt = meta["ntiles"][rel]
        ngrp = _cdiv(nt, OUT_GRP)
        for c in range(NCORES):
            ids = meta["dst_ids"][rel][c]
            m = ids >= 0
            arr = np.asarray(results[c][names[rel]], np.float32)
            rows = arr.reshape(ngrp, 128, OUT_GRP, HID).transpose(
                0, 2, 1, 3).reshape(ngrp * OUT_GRP * 128, HID)[:nt * 128]
            out[offs[rel] + ids[m]] = rows[m]
    return out


def kernel(**inputs):
    import hashlib
    key = "prog"
    h = hashlib.sha1()
    for k in ("sc_src", "sc_dst", "in_src", "in_dst", "ni_src", "ni_dst"):
        h.update(np.ascontiguousarray(np.asarray(inputs[k], np.int32)).tobytes())
    sig = h.hexdigest()
    meta, in_maps = _build_host_data(inputs)
    if key in _cache and _cache[key][0] == sig:
        _, nc, _ = _cache[key]
    else:
        nc = _build_program(meta)
        _cache[key] = (sig, nc, meta)
    res = _run(nc, in_maps)
    return _assemble(res.results, meta)


# revision 63
# speedup vs baseline: 1.0648x; 1.0095x over previous
"""Trainium2 Bass kernel for a heterogeneous GraphConv layer (3 relations).

out = concat([leaky(GC(inst_feat, W_inst, in_*)),     # -> node   (10000)
              leaky(GC(node_feat, W_node, ni_*)),     # -> inst   (100000)
              leaky(GC(svc_feat,  W_svc,  sc_*))])    # -> svc    (20000)

GC(f, W, src, dst) = rsqrt(deg_d) * segsum_dst((rsqrt(deg_s)*f)[src]) @ W + b
(aggregation commutes with the dense @W, so we gather *raw scaled features*
and apply W once per destination tile).

Strategy: destination-sharded across 8 NeuronCores, with host-balanced
dst->(core,tile,slot) assignment (tile edge sums packed to multiples of 128
and rank-matched across cores so the shared program's per-tile block counts
stay near the mean).

DMA-gather cost on TRN2 is per-descriptor: a 512B descriptor costs the same
as 256B (sub-512B transfers pay a 2x bus penalty), so the gather tables are
laid out as PAIR rows [2*128] fp16 = 512B: one descriptor fetches two
feature rows. The per-core table is ordered by first use so the two edges
that introduce a pair of new sources in the same dst tile share one
descriptor ("paired slots"); repeat edges use one half and the other half
rides free (and occasionally serves a second edge whose source lands on the
pair sibling). Per slot there are two one-hot channels dlA/dlB (-1 = unused)
selecting the dst column for the low/high half row.

Device work per 128-slot block: dma_gather 128 pair rows -> [128e, 256f],
DVE tensor_scalar builds one-hot stA[e,d]=(iota==dlA) (and stB for the
pair-slot prefix blocks only), PE accumulates aggT[f,d] += A.T@stA (+B.T@stB)
in PSUM. Per dst tile: PSUM out = u (x) b (rank-1 K=1 fp32 matmul preloading
bias) + aggT.T @ W (fp16), then one ScalarE Lrelu(out * rsqrt_deg_d) and a
grouped fp16 DMA to the output rows.
"""

import os as _os

import numpy as np

SVC_N, INST_N, NODE_N, HID = 20000, 100000, 10000, 128
NCORES = 8
BLK = 128           # slots per one-hot matmul block
# blocks per dma_gather instruction. NOTE: needs single_packet=False — with
# single_packet=True the whole stream coalesces into one DMA packet, which
# caps at 64 descriptors/engine (num_idxs <= 1024); beyond that the exec
# unit faults (NRT_EXEC_UNIT_UNRECOVERABLE).
CHUNK = int(_os.environ.get("GNN_CHUNK", "16"))
GDT = "fp16"
ACT_MODE = "lrelu"  # "lrelu" (HW leaky relu) | "relu" (sim debug)
PAIR = 2 * HID      # table row = pair of feature rows (512B fp16)
# dst tiles batched per epilogue staging buffer / out DMA
OUT_GRP = int(_os.environ.get("GNN_OUT_GRP", "16"))

_cache = {}


def _cdiv(a, b):
    return (a + b - 1) // b


def _rup(a, b):
    return _cdiv(a, b) * b


def _balanced_assign(deg, n_dst, ntiles, rho):
    """Assign dst nodes to (core, tile, slot) packing per-tile SLOT counts
    (estimated as edges*(1-rho), rho = relation merge rate) just under
    multiples of 128 so per-tile block counts carry minimal ceil padding.

    Returns dst_ids[NCORES, ntiles*128] int64 (-1 = pad slot): the global dst
    node stored at each (core, tile, slot).
    """
    order = np.argsort(-deg, kind="stable")
    # snake over cores -> per-core totals equal to within one max-degree
    core_of = np.empty(n_dst, np.int64)
    snake = np.concatenate([np.arange(NCORES), np.arange(NCORES)[::-1]])
    core_of[order] = snake[np.arange(n_dst) % (2 * NCORES)]

    dst_ids = np.full((NCORES, ntiles * 128), -1, np.int64)
    for c in range(NCORES):
        ids = order[core_of[order] == c]  # degree-descending
        w = deg[ids]
        total = int(w.sum())
        # uniform edge-weight target per bin (multiple of 128; rho reserved
        # for a future slot-aware quota scheme — measured merge-rate noise
        # and the max-over-core coupling made per-bin quotas regress)
        target = np.full(ntiles, _cdiv(total, ntiles * 128) * 128, np.int64)
        binw = np.zeros(ntiles, np.int64)
        binn = np.zeros(ntiles, np.int64)
        bins = [[] for _ in range(ntiles)]
        # greedy: place each dst (deg desc) in the fullest bin it still fits
        # (by weight target and 128-slot cap); else least-filled open bin
        open_bins = list(range(ntiles))
        for i, d in zip(ids, w):
            best, bestw = -1, -1
            for t in open_bins:
                if binw[t] + d <= target[t] and binw[t] > bestw:
                    best, bestw = t, binw[t]
            if best < 0:
                best = min(open_bins, key=lambda x: binw[x])
            t = best
            bins[t].append(i)
            binw[t] += d
            binn[t] += 1
            if binn[t] >= 128:
                open_bins.remove(t)
        for t in range(ntiles):
            ids_t = bins[t]
            dst_ids[c, t * 128: t * 128 + len(ids_t)] = ids_t
    return dst_ids


def _pack_core(es, ed, n_src, ntiles):
    """Pair-slot packing for one (core, relation).

    es: edge source node ids; ed: edge dst slot (tile*128 + dst_local).

    The gather table holds one 512B row per USED source: [feat(r), feat(r+1)]
    (staggered duplicate), so descriptor idx r serves edge(s) on table row r
    via channel A and optionally a second edge on row r+1 via channel B.
    Rows are ordered by their tile-usage lists (lexsort) so edges of the same
    tile sit on adjacent rows and merge into shared slots.

    Returns dict with: table_rows (src id per table row), per-tile slot
    arrays (row idx, dlA, dlB), nslot[t], nB[t].
    """
    KEYLEN = 6
    tile_of = (ed >> 7).astype(np.int64)
    dl = (ed & 127).astype(np.int64)

    rows_used = np.unique(es)
    nrows = len(rows_used)
    rid_of = np.full(n_src, -1, np.int64)
    rid_of[rows_used] = np.arange(nrows)

    # tile-usage key per row: first KEYLEN tiles (sorted), padded
    pt = np.unique(np.stack([rid_of[es], tile_of], axis=1), axis=0)
    grp_new = np.r_[True, pt[1:, 0] != pt[:-1, 0]]
    idx = np.arange(len(pt))
    j = idx - np.maximum.accumulate(np.where(grp_new, idx, 0))
    keymat = np.full((nrows, KEYLEN), 32767, np.int64)
    m = j < KEYLEN
    keymat[pt[m, 0], j[m]] = pt[m, 1]
    order = np.lexsort(keymat.T[::-1])
    table_rows = rows_used[order]          # src id at each table position
    pos_of = np.full(n_src, -1, np.int64)
    pos_of[table_rows] = np.arange(nrows)

    r = pos_of[es]
    # per tile: sort edges by table position; pair edges on consecutive
    # positions (runs split on gaps/duplicates, paired (0,1)(2,3)... in-run)
    o = np.lexsort((r, tile_of))
    kt, kr, kdl = tile_of[o], r[o], dl[o]
    brk = np.r_[True, (kt[1:] != kt[:-1]) | (kr[1:] != kr[:-1] + 1)]
    idx = np.arange(len(kt))
    k_in_run = idx - np.maximum.accumulate(np.where(brk, idx, 0))
    run_id = np.cumsum(brk) - 1
    half = k_in_run & 1
    skey = np.stack([run_id, k_in_run >> 1], axis=1)
    uslot, inv = np.unique(skey, axis=0, return_inverse=True)
    ns = len(uslot)
    s_tile = np.zeros(ns, np.int64)
    s_row = np.zeros(ns, np.int64)
    dlA = np.full(ns, -1, np.int64)
    dlB = np.full(ns, -1, np.int64)
    m0 = half == 0
    s_tile[inv[m0]] = kt[m0]
    s_row[inv[m0]] = kr[m0]
    dlA[inv[m0]] = kdl[m0]
    dlB[inv[~m0]] = kdl[~m0]
    has_b = dlB >= 0

    # order slots per tile: B-present first (so stB/matmul-B run only on a
    # prefix of blocks), then by row for gather locality
    so = np.lexsort((s_row, ~has_b, s_tile))
    s_tile, s_row, dlA, dlB, has_b = (
        s_tile[so], s_row[so], dlA[so], dlB[so], has_b[so])

    nslot = np.bincount(s_tile, minlength=ntiles)
    nB = np.bincount(s_tile[has_b], minlength=ntiles)
    tstart = np.r_[0, np.cumsum(nslot)]
    tiles = []
    for t in range(ntiles):
        sl = slice(tstart[t], tstart[t + 1])
        tiles.append((s_row[sl], dlA[sl], dlB[sl]))
    return dict(table_rows=table_rows, tiles=tiles, nslot=nslot, nB=nB)


def _prep_relation(src, dst, n_src, n_dst, feat, rho):
    """Host-side sharding/packing for one relation.

    rho: estimated slot merge rate (pair-served edge fraction) used to pack
    tiles to near-multiple-of-128 slot counts.
    """
    src = np.asarray(src, np.int64)
    dst = np.asarray(dst, np.int64)
    deg_s = np.maximum(np.bincount(src, minlength=n_src), 1).astype(np.float64)
    deg_d_raw = np.bincount(dst, minlength=n_dst)
    deg_d = np.maximum(deg_d_raw, 1).astype(np.float64)
    rs_s = (1.0 / np.sqrt(deg_s)).astype(np.float32)
    rs_d = (1.0 / np.sqrt(deg_d)).astype(np.float32)
    u_d = np.sqrt(deg_d).astype(np.float32)  # ~= 1/rs_d

    feat_s = (np.asarray(feat, np.float32) * rs_s[:, None]).astype(np.float32)

    D = _rup(_cdiv(n_dst, NCORES), 128)  # dst rows per core (padded)
    ntiles = D // 128

    dst_ids = _balanced_assign(deg_d_raw.astype(np.int64), n_dst, ntiles, rho)
    slot_core = np.empty(n_dst, np.int64)
    slot_loc = np.empty(n_dst, np.int64)
    for c in range(NCORES):
        m = dst_ids[c] >= 0
        slot_core[dst_ids[c, m]] = c
        slot_loc[dst_ids[c, m]] = np.nonzero(m)[0]

    e_core = slot_core[dst]
    e_loc = slot_loc[dst]
    cores = []
    for c in range(NCORES):
        m = e_core == c
        pk = _pack_core(src[m], e_loc[m], n_src, ntiles)
        pk["dst_ids"] = dst_ids[c].copy()
        cores.append(pk)

    # rank-match: per core sort its tiles by slot count desc so tile index t
    # has similar (max-over-core) block counts
    for c in range(NCORES):
        pk = cores[c]
        perm = np.argsort(-pk["nslot"], kind="stable")
        pk["tiles"] = [pk["tiles"][t] for t in perm]
        pk["nslot"] = pk["nslot"][perm]
        pk["nB"] = pk["nB"][perm]
        pk["dst_ids"] = pk["dst_ids"].reshape(ntiles, 128)[perm].reshape(-1)

    nslot_all = np.stack([cores[c]["nslot"] for c in range(NCORES)])
    nB_all = np.stack([cores[c]["nB"] for c in range(NCORES)])
    btile = np.maximum(_cdiv(np.max(nslot_all, axis=0), BLK), 1)
    nbtile = np.minimum(_cdiv(np.max(nB_all, axis=0), BLK), btile)
    nrows = max(len(cores[c]["table_rows"]) for c in range(NCORES))

    return dict(cores=cores, btile=btile, nbtile=nbtile, nrows=nrows,
                nblk=int(btile.sum()), ntiles=ntiles, D=D,
                feat_s=feat_s, rs_d=rs_d, u_d=u_d, n_dst=n_dst)


def _build_host_data(inputs):
    rels = [
        # order matters: output rows are [node_out, inst_out, svc_out].
        # rho = measured pair-merge rate per relation on this graph
        _prep_relation(inputs["in_src"], inputs["in_dst"], INST_N, NODE_N,
                       inputs["instance_feat"], rho=0.49),
        _prep_relation(inputs["ni_src"], inputs["ni_dst"], NODE_N, INST_N,
                       inputs["node_feat"], rho=0.30),
        _prep_relation(inputs["sc_src"], inputs["sc_dst"], SVC_N, SVC_N,
                       inputs["svc_feat"], rho=0.41),
    ]
    Ws = [inputs["W_inst"], inputs["W_node"], inputs["W_svc"]]
    bs = [inputs["b_inst"], inputs["b_node"], inputs["b_svc"]]

    # per-relation gather chunk size minimizing relation-tail pad blocks
    # (pad descriptors are charged by the DMA model even when reg-trimmed)
    def _best_chunk(nblk):
        return min(range(15, 25),
                   key=lambda cc: (_rup(nblk, cc) - nblk, abs(cc - CHUNK)))

    chunks = [_best_chunk(r["nblk"]) for r in rels]
    nblk_pads = [_rup(r["nblk"], chunks[i]) for i, r in enumerate(rels)]
    nblk_tot = sum(nblk_pads)
    nidx_tot = nblk_tot * BLK
    ntile_tot = sum(r["ntiles"] for r in rels)

    W_cat = np.concatenate([np.asarray(w, np.float32) for w in Ws],
                           axis=1).astype(np.float16)
    b_row = np.concatenate([np.asarray(b, np.float32) for b in bs]
                           )[None, :].astype(np.float16)
    iota = np.tile(np.arange(128, dtype=np.float32), (128, 1)).astype(np.float16)

    in_maps = []
    for c in range(NCORES):
        gidx = np.full(nidx_tot, -1, np.int64)
        dlA = np.full(nidx_tot, -1.0, np.float32)
        dlB = np.full(nidx_tot, -1.0, np.float32)
        tbls = []
        rel_bases = np.r_[0, np.cumsum(nblk_pads)]
        for ri, r in enumerate(rels):
            off = int(rel_bases[ri])  # block offset in global stream
            pk = r["cores"][c]
            assert r["nrows"] < 32768, "row idx must fit int16"
            # staggered-duplicate pair rows: tbl[i] = [feat(i), feat(i+1)]
            fr = np.zeros((r["nrows"] + 1, HID), np.float16)
            tr = pk["table_rows"]
            fr[:len(tr)] = r["feat_s"][tr].astype(np.float16)
            tbl = np.concatenate([fr[:-1], fr[1:]], axis=1)
            tbls.append(np.ascontiguousarray(tbl))
            for t in range(r["ntiles"]):
                sp, da, db = pk["tiles"][t]
                n = len(sp)
                base = off * BLK
                gidx[base:base + n] = sp
                gidx[base + n: base + int(r["btile"][t]) * BLK] = 0
                dlA[base:base + n] = da
                dlB[base:base + n] = db
                off += int(r["btile"][t])
            # relation-tail pad blocks keep idx -1 (trimmed device-side)

        idx16 = np.ascontiguousarray(
            gidx.astype(np.int16).reshape(-1, 16).T)
        idx_sb = np.tile(idx16, (8, 1))                          # [128, nidx/16]
        dlA_sb = np.ascontiguousarray(dlA.reshape(nblk_tot, BLK).T)
        dlB_sb = np.ascontiguousarray(dlB.reshape(nblk_tot, BLK).T)

        rs_sb = np.zeros((128, ntile_tot), np.float32)
        u_sb = np.zeros((1, ntile_tot * 128), np.float32)
        t0 = 0
        for r in rels:
            ids = r["cores"][c]["dst_ids"]
            val_rs = np.zeros(r["D"], np.float32)
            val_u = np.zeros(r["D"], np.float32)
            m = ids >= 0
            val_rs[m] = r["rs_d"][ids[m]]
            val_u[m] = r["u_d"][ids[m]]
            rs_sb[:, t0:t0 + r["ntiles"]] = val_rs.reshape(r["ntiles"], 128).T
            u_sb[0, t0 * 128:(t0 + r["ntiles"]) * 128] = val_u
            t0 += r["ntiles"]
        u_sb = u_sb.astype(np.float16)

        in_maps.append({
            "tbl_in": tbls[0],
            "tbl_ni": tbls[1],
            "tbl_sc": tbls[2],
            "idx_sb": np.ascontiguousarray(idx_sb),
            "dlA_sb": dlA_sb,
            "dlB_sb": dlB_sb,
            "rs_sb": rs_sb,
            "u_sb": u_sb,
            "W_cat": np.ascontiguousarray(W_cat),
            "b_row": np.ascontiguousarray(b_row),
            "iota": np.ascontiguousarray(iota),
        })

    meta = dict(
        chunks=chunks,
        has_bias=bool(np.any(b_row != 0)),
        nblk_tot=nblk_tot, nidx_tot=nidx_tot, ntile_tot=ntile_tot,
        nrowss=[r["nrows"] for r in rels],
        btiles=[r["btile"].tolist() for r in rels],
        nbtiles=[r["nbtile"].tolist() for r in rels],
        ntiles=[r["ntiles"] for r in rels],
        Ds=[r["D"] for r in rels],
        n_dsts=[r["n_dst"] for r in rels],
        dst_ids=[[r["cores"][c]["dst_ids"] for c in range(NCORES)]
                 for r in rels],
    )
    return meta, in_maps


def _build_program(meta):
    import os

    import concourse.bacc as bacc
    import concourse.mybir as mybir
    import concourse.tile as tile

    dbg_max_tiles = int(os.environ.get("GNN_MAX_TILES", "0"))  # 0 = all

    gdt = mybir.dt.float16
    f16 = mybir.dt.float16
    f32 = mybir.dt.float32
    AF = mybir.ActivationFunctionType
    act_fn = AF.Lrelu if ACT_MODE == "lrelu" else AF.Relu

    nblk_tot, nidx_tot, ntile_tot = (meta["nblk_tot"], meta["nidx_tot"],
                                     meta["ntile_tot"])

    nc = bacc.Bacc("TRN2", target_bir_lowering=False, debug=False,
                   enable_asserts=False, num_devices=NCORES)

    tbl_d = [
        nc.dram_tensor("tbl_in", [meta["nrowss"][0], PAIR], gdt,
                       kind="ExternalInput"),
        nc.dram_tensor("tbl_ni", [meta["nrowss"][1], PAIR], gdt,
                       kind="ExternalInput"),
        nc.dram_tensor("tbl_sc", [meta["nrowss"][2], PAIR], gdt,
                       kind="ExternalInput"),
    ]
    idx_d = nc.dram_tensor("idx_sb", [128, nidx_tot // 16], mybir.dt.int16,
                           kind="ExternalInput")
    dlA_d = nc.dram_tensor("dlA_sb", [128, nblk_tot], f32, kind="ExternalInput")
    dlB_d = nc.dram_tensor("dlB_sb", [128, nblk_tot], f32, kind="ExternalInput")
    rs_d = nc.dram_tensor("rs_sb", [128, ntile_tot], f32, kind="ExternalInput")
    u_d = nc.dram_tensor("u_sb", [1, ntile_tot * 128], f16, kind="ExternalInput")
    W_d = nc.dram_tensor("W_cat", [128, 3 * HID], f16, kind="ExternalInput")
    b_d = nc.dram_tensor("b_row", [1, 3 * HID], f16, kind="ExternalInput")
    iota_d = nc.dram_tensor("iota", [128, 128], gdt, kind="ExternalInput")

    # p-major grouped layout: row g*128+p holds OUT_GRP tiles' rows for dst
    # slot p — out DMA descriptors are OUT_GRP*256B contiguous (no sub-512B
    # DMA bus penalty); host assemble untangles
    out_d = [
        nc.dram_tensor(n, [_cdiv(meta["ntiles"][i], OUT_GRP) * 128,
                           OUT_GRP * HID], f16, kind="ExternalOutput")
        for i, n in enumerate(["out_node", "out_inst", "out_svc"])
    ]

    with tile.TileContext(nc) as tc:
        with (
            tc.tile_pool(name="const", bufs=1) as const,
            tc.tile_pool(name="g", bufs=int(os.environ.get("GNN_GBUFS", "7"))) as gpool,
            tc.tile_pool(name="st", bufs=int(os.environ.get("GNN_STBUFS", "48"))) as stpool,
            tc.tile_pool(name="evac", bufs=int(os.environ.get("GNN_EVBUFS", "6"))) as evac,
            tc.tile_pool(name="osb", bufs=int(os.environ.get("GNN_OBUFS", "6"))) as opool,
            tc.tile_pool(name="psA", bufs=5, space="PSUM") as psA,
            tc.tile_pool(name="psO", bufs=3, space="PSUM") as psO,
        ):
            rel_nblks = [sum(meta["btiles"][r]) for r in range(3)]
            rel_base = [0, 0, 0]  # global block base per relation
            chs = meta["chunks"]
            for r in range(1, 3):
                rel_base[r] = rel_base[r - 1] + _rup(rel_nblks[r - 1],
                                                     chs[r - 1])
            tg_base = [0, meta["ntiles"][0],
                       meta["ntiles"][0] + meta["ntiles"][1]]

            # first gathers depend only on the leading idx slices + dl/iota:
            # load those first so the gather stream starts ASAP (HWDGE is
            # FIFO per issuing engine)
            idx_t = const.tile([128, nidx_tot // 16], mybir.dt.int16)
            heads = []
            for r in range(3):
                h0 = rel_base[r] * BLK // 16
                h1 = min(h0 + 2 * chs[r] * BLK // 16, nidx_tot // 16)
                heads.append((h0, h1))
                nc.sync.dma_start(idx_t[:, h0:h1], idx_d.ap()[:, h0:h1])
            dlA_t = const.tile([128, nblk_tot], f32)
            nc.sync.dma_start(dlA_t[:], dlA_d.ap())
            dlB_t = const.tile([128, nblk_tot], f32)
            nc.sync.dma_start(dlB_t[:], dlB_d.ap())
            iota_t = const.tile([128, 128], gdt)
            nc.sync.dma_start(iota_t[:], iota_d.ap())
            W_t = const.tile([128, 3 * HID], f16)
            nc.sync.dma_start(W_t[:], W_d.ap())
            b_t = const.tile([1, 3 * HID], f16)
            nc.sync.dma_start(b_t[:], b_d.ap())
            u_t = const.tile([1, ntile_tot * 128], f16)
            nc.sync.dma_start(u_t[:], u_d.ap())
            rs_t = const.tile([128, ntile_tot], f32)
            nc.sync.dma_start(rs_t[:], rs_d.ap())
            for r in range(3):
                h1 = heads[r][1]
                end = rel_base[r] * BLK // 16 + \
                    _rup(rel_nblks[r], chs[r]) * BLK // 16
                if h1 < end:
                    nc.sync.dma_start(idx_t[:, h1:end], idx_d.ap()[:, h1:end])

            g_tiles = {}   # (rel, rel-local chunk) -> gather tile

            def issue_gather(rel, lci):
                ch = chs[rel]
                # last chunk: shrink to the real remainder so the cost-model
                # (and HW) never touches relation-tail pad descriptors
                real_blocks = max(1, min(ch, rel_nblks[rel] - lci * ch))
                gt = gpool.tile([128, real_blocks, PAIR], gdt, tag="g",
                                name="gt")
                nidx = real_blocks * BLK
                h0 = rel_base[rel] * BLK // 16 + lci * (ch * BLK // 16)
                nc.gpsimd.dma_gather(
                    out_ap=gt[:],
                    in_ap=tbl_d[rel].ap(),
                    idxs_ap=idx_t[:, h0:h0 + nidx // 16],
                    num_idxs=nidx,
                    num_idxs_reg=nidx,
                    elem_size=PAIR,
                    single_packet=False,
                )
                g_tiles[(rel, lci)] = gt

            # relation-interleaved tile schedule: spread the epilogue-heavy
            # relation (ni: many tiles, few blocks) evenly across the
            # gather-heavy one (sc) so no engine's work bunches up
            sched = []
            scale = [0.93, 0.95, 1.0]  # in/ni finish early; sc's last blocks
            for r in range(3):         # keep gathers flowing during drain
                btile = meta["btiles"][r]
                o = 0
                for t in range(meta["ntiles"][r]):
                    if dbg_max_tiles and t >= dbg_max_tiles:
                        break
                    # key on end-fraction: the block-heavy relation's last
                    # tile keeps gathers flowing while light tiles' epilogues
                    # drain, shrinking the no-DMA tail
                    sched.append((scale[r] * (o + btile[t])
                                  / max(1, rel_nblks[r]), r, t))
                    o += btile[t]
            sched.sort()

            rel_blk = [0, 0, 0]       # relation-local block cursor
            osb_state = [None, None, None]

            for _, rel, t in sched:
                ntiles = meta["ntiles"][rel]
                nb = meta["btiles"][rel][t]
                nbB = meta["nbtiles"][rel][t]
                agg = psA.tile([128, 128], f32, tag="agg")
                for b in range(nb):
                    lb = rel_blk[rel]
                    lci, cj = divmod(lb, chs[rel])
                    if cj == 0:
                        issue_gather(rel, lci)
                    blk = rel_base[rel] + lb  # global dl column
                    has_b = b < nbB
                    stA = stpool.tile([128, 128], gdt, tag="stA")
                    nc.vector.tensor_scalar(
                        stA[:], iota_t[:], dlA_t[:, blk:blk + 1], None,
                        mybir.AluOpType.is_equal)
                    last = b == nb - 1
                    nc.tensor.matmul(
                        agg[:], g_tiles[(rel, lci)][:, cj, 0:HID], stA[:],
                        start=(b == 0), stop=(last and not has_b))
                    if has_b:
                        stB = stpool.tile([128, 128], gdt, tag="stB")
                        nc.vector.tensor_scalar(
                            stB[:], iota_t[:], dlB_t[:, blk:blk + 1], None,
                            mybir.AluOpType.is_equal)
                        nc.tensor.matmul(
                            agg[:], g_tiles[(rel, lci)][:, cj, HID:PAIR], stB[:],
                            start=False, stop=last)
                    rel_blk[rel] += 1
                tglob = tg_base[rel] + t
                aggsb = evac.tile([128, 128], f16, tag="evac")
                nc.scalar.copy(aggsb[:], agg[:])
                po = psO.tile([128, 128], f32, tag="po")
                if meta["has_bias"]:
                    nc.tensor.matmul(
                        po[:], u_t[:, tglob * 128:(tglob + 1) * 128],
                        b_t[:, rel * HID:(rel + 1) * HID],
                        start=True, stop=False, skip_group_check=True)
                nc.tensor.matmul(
                    po[:], aggsb[:], W_t[:, rel * HID:(rel + 1) * HID],
                    start=not meta["has_bias"], stop=True,
                    skip_group_check=True)
                oj = t % OUT_GRP
                if oj == 0:
                    osb_state[rel] = (
                        opool.tile([128, OUT_GRP, 128], f16, tag="osb",
                                   name="osb"), t)
                osb, osb_t0 = osb_state[rel]
                nc.scalar.activation(
                    osb[:, oj, :], po[:], act_fn,
                    bias=0.0, scale=rs_t[:, tglob:tglob + 1], alpha=0.01)
                if oj == OUT_GRP - 1 or t == ntiles - 1:
                    cnt = t - osb_t0 + 1
                    g = osb_t0 // OUT_GRP
                    dst = out_d[rel].ap()[g * 128:(g + 1) * 128, :cnt * HID]
                    nc.sync.dma_start(
                        dst.rearrange("p (j k) -> p j k", k=HID),
                        osb[:, :cnt, :])

    nc.compile()
    return nc


def _run(nc, in_maps, trace=False, **kw):
    from concourse import bass_utils
    res = bass_utils.run_bass_kernel_spmd(
        nc, in_maps, core_ids=list(range(NCORES)), trace=trace, **kw)
    return res


def _assemble(results, meta):
    out = np.empty((NODE_N + INST_N + SVC_N, HID), np.float32)
    offs = [0, NODE_N, NODE_N + INST_N]
    names = ["out_node", "out_inst", "out_svc"]
    for rel in range(3):
        nt = meta["ntiles"][rel]
        ngrp = _cdiv(nt, OUT_GRP)
        for c in range(NCORES):
            ids = meta["dst_ids"][rel][c]
            m = ids >= 0
            arr = np.asarray(results[c][names[rel]], np.float32)
            rows = arr.reshape(ngrp, 128, OUT_GRP, HID).transpose(
                0, 2, 1, 3).reshape(ngrp * OUT_GRP * 128, HID)[:nt * 128]
            out[offs[rel] + ids[m]] = rows[m]
    return out


def kernel(**inputs):
    import hashlib
    key = "prog"
    h = hashlib.sha1()
    for k in ("sc_src", "sc_dst", "in_src", "in_dst", "ni_src", "ni_dst"):
        h.update(np.ascontiguousarray(np.asarray(inputs[k], np.int32)).tobytes())
    sig = h.hexdigest()
    meta, in_maps = _build_host_data(inputs)
    if key in _cache and _cache[key][0] == sig:
        _, nc, _ = _cache[key]
    else:
        nc = _build_program(meta)
        _cache[key] = (sig, nc, meta)
    res = _run(nc, in_maps)
    return _assemble(res.results, meta)


# revision 64
# speedup vs baseline: 1.0702x; 1.0050x over previous
"""Trainium2 Bass kernel for a heterogeneous GraphConv layer (3 relations).

out = concat([leaky(GC(inst_feat, W_inst, in_*)),     # -> node   (10000)
              leaky(GC(node_feat, W_node, ni_*)),     # -> inst   (100000)
              leaky(GC(svc_feat,  W_svc,  sc_*))])    # -> svc    (20000)

GC(f, W, src, dst) = rsqrt(deg_d) * segsum_dst((rsqrt(deg_s)*f)[src]) @ W + b
(aggregation commutes with the dense @W, so we gather *raw scaled features*
and apply W once per destination tile).

Strategy: destination-sharded across 8 NeuronCores, with host-balanced
dst->(core,tile,slot) assignment (tile edge sums packed to multiples of 128
and rank-matched across cores so the shared program's per-tile block counts
stay near the mean).

DMA-gather cost on TRN2 is per-descriptor: a 512B descriptor costs the same
as 256B (sub-512B transfers pay a 2x bus penalty), so the gather tables are
laid out as PAIR rows [2*128] fp16 = 512B: one descriptor fetches two
feature rows. The per-core table is ordered by first use so the two edges
that introduce a pair of new sources in the same dst tile share one
descriptor ("paired slots"); repeat edges use one half and the other half
rides free (and occasionally serves a second edge whose source lands on the
pair sibling). Per slot there are two one-hot channels dlA/dlB (-1 = unused)
selecting the dst column for the low/high half row.

Device work per 128-slot block: dma_gather 128 pair rows -> [128e, 256f],
DVE tensor_scalar builds one-hot stA[e,d]=(iota==dlA) (and stB for the
pair-slot prefix blocks only), PE accumulates aggT[f,d] += A.T@stA (+B.T@stB)
in PSUM. Per dst tile: PSUM out = u (x) b (rank-1 K=1 fp32 matmul preloading
bias) + aggT.T @ W (fp16), then one ScalarE Lrelu(out * rsqrt_deg_d) and a
grouped fp16 DMA to the output rows.
"""

import os as _os

import numpy as np

SVC_N, INST_N, NODE_N, HID = 20000, 100000, 10000, 128
NCORES = 8
BLK = 128           # slots per one-hot matmul block
# blocks per dma_gather instruction. NOTE: needs single_packet=False — with
# single_packet=True the whole stream coalesces into one DMA packet, which
# caps at 64 descriptors/engine (num_idxs <= 1024); beyond that the exec
# unit faults (NRT_EXEC_UNIT_UNRECOVERABLE).
CHUNK = int(_os.environ.get("GNN_CHUNK", "16"))
GDT = "fp16"
ACT_MODE = "lrelu"  # "lrelu" (HW leaky relu) | "relu" (sim debug)
PAIR = 2 * HID      # table row = pair of feature rows (512B fp16)
# dst tiles batched per epilogue staging buffer / out DMA
OUT_GRP = int(_os.environ.get("GNN_OUT_GRP", "16"))

_cache = {}


def _cdiv(a, b):
    return (a + b - 1) // b


def _rup(a, b):
    return _cdiv(a, b) * b


def _balanced_assign(deg, n_dst, ntiles, rho):
    """Assign dst nodes to (core, tile, slot) packing per-tile SLOT counts
    (estimated as edges*(1-rho), rho = relation merge rate) just under
    multiples of 128 so per-tile block counts carry minimal ceil padding.

    Returns dst_ids[NCORES, ntiles*128] int64 (-1 = pad slot): the global dst
    node stored at each (core, tile, slot).
    """
    order = np.argsort(-deg, kind="stable")
    # snake over cores -> per-core totals equal to within one max-degree
    core_of = np.empty(n_dst, np.int64)
    snake = np.concatenate([np.arange(NCORES), np.arange(NCORES)[::-1]])
    core_of[order] = snake[np.arange(n_dst) % (2 * NCORES)]

    dst_ids = np.full((NCORES, ntiles * 128), -1, np.int64)
    for c in range(NCORES):
        ids = order[core_of[order] == c]  # degree-descending
        w = deg[ids]
        total = int(w.sum())
        # uniform edge-weight target per bin (multiple of 128; rho reserved
        # for a future slot-aware quota scheme — measured merge-rate noise
        # and the max-over-core coupling made per-bin quotas regress)
        target = np.full(ntiles, _cdiv(total, ntiles * 128) * 128, np.int64)
        binw = np.zeros(ntiles, np.int64)
        binn = np.zeros(ntiles, np.int64)
        bins = [[] for _ in range(ntiles)]
        # greedy: place each dst (deg desc) in the fullest bin it still fits
        # (by weight target and 128-slot cap); else least-filled open bin
        open_bins = list(range(ntiles))
        for i, d in zip(ids, w):
            best, bestw = -1, -1
            for t in open_bins:
                if binw[t] + d <= target[t] and binw[t] > bestw:
                    best, bestw = t, binw[t]
            if best < 0:
                best = min(open_bins, key=lambda x: binw[x])
            t = best
            bins[t].append(i)
            binw[t] += d
            binn[t] += 1
            if binn[t] >= 128:
                open_bins.remove(t)
        for t in range(ntiles):
            ids_t = bins[t]
            dst_ids[c, t * 128: t * 128 + len(ids_t)] = ids_t
    return dst_ids


def _pack_core(es, ed, n_src, ntiles):
    """Pair-slot packing for one (core, relation).

    es: edge source node ids; ed: edge dst slot (tile*128 + dst_local).

    The gather table holds one 512B row per USED source: [feat(r), feat(r+1)]
    (staggered duplicate), so descriptor idx r serves edge(s) on table row r
    via channel A and optionally a second edge on row r+1 via channel B.
    Rows are ordered by their tile-usage lists (lexsort) so edges of the same
    tile sit on adjacent rows and merge into shared slots.

    Returns dict with: table_rows (src id per table row), per-tile slot
    arrays (row idx, dlA, dlB), nslot[t], nB[t].
    """
    KEYLEN = 6
    tile_of = (ed >> 7).astype(np.int64)
    dl = (ed & 127).astype(np.int64)

    rows_used = np.unique(es)
    nrows = len(rows_used)
    rid_of = np.full(n_src, -1, np.int64)
    rid_of[rows_used] = np.arange(nrows)

    # tile-usage key per row: first KEYLEN tiles (sorted), padded
    pt = np.unique(np.stack([rid_of[es], tile_of], axis=1), axis=0)
    grp_new = np.r_[True, pt[1:, 0] != pt[:-1, 0]]
    idx = np.arange(len(pt))
    j = idx - np.maximum.accumulate(np.where(grp_new, idx, 0))
    keymat = np.full((nrows, KEYLEN), 32767, np.int64)
    m = j < KEYLEN
    keymat[pt[m, 0], j[m]] = pt[m, 1]
    order = np.lexsort(keymat.T[::-1])
    table_rows = rows_used[order]          # src id at each table position
    pos_of = np.full(n_src, -1, np.int64)
    pos_of[table_rows] = np.arange(nrows)

    r = pos_of[es]
    # per tile: sort edges by table position; pair edges on consecutive
    # positions (runs split on gaps/duplicates, paired (0,1)(2,3)... in-run)
    o = np.lexsort((r, tile_of))
    kt, kr, kdl = tile_of[o], r[o], dl[o]
    brk = np.r_[True, (kt[1:] != kt[:-1]) | (kr[1:] != kr[:-1] + 1)]
    idx = np.arange(len(kt))
    k_in_run = idx - np.maximum.accumulate(np.where(brk, idx, 0))
    run_id = np.cumsum(brk) - 1
    half = k_in_run & 1
    skey = np.stack([run_id, k_in_run >> 1], axis=1)
    uslot, inv = np.unique(skey, axis=0, return_inverse=True)
    ns = len(uslot)
    s_tile = np.zeros(ns, np.int64)
    s_row = np.zeros(ns, np.int64)
    dlA = np.full(ns, -1, np.int64)
    dlB = np.full(ns, -1, np.int64)
    m0 = half == 0
    s_tile[inv[m0]] = kt[m0]
    s_row[inv[m0]] = kr[m0]
    dlA[inv[m0]] = kdl[m0]
    dlB[inv[~m0]] = kdl[~m0]
    has_b = dlB >= 0

    # order slots per tile: B-present first (so stB/matmul-B run only on a
    # prefix of blocks), then by row for gather locality
    so = np.lexsort((s_row, ~has_b, s_tile))
    s_tile, s_row, dlA, dlB, has_b = (
        s_tile[so], s_row[so], dlA[so], dlB[so], has_b[so])

    nslot = np.bincount(s_tile, minlength=ntiles)
    nB = np.bincount(s_tile[has_b], minlength=ntiles)
    tstart = np.r_[0, np.cumsum(nslot)]
    tiles = []
    for t in range(ntiles):
        sl = slice(tstart[t], tstart[t + 1])
        tiles.append((s_row[sl], dlA[sl], dlB[sl]))
    return dict(table_rows=table_rows, tiles=tiles, nslot=nslot, nB=nB)


def _prep_relation(src, dst, n_src, n_dst, feat, rho):
    """Host-side sharding/packing for one relation.

    rho: estimated slot merge rate (pair-served edge fraction) used to pack
    tiles to near-multiple-of-128 slot counts.
    """
    src = np.asarray(src, np.int64)
    dst = np.asarray(dst, np.int64)
    deg_s = np.maximum(np.bincount(src, minlength=n_src), 1).astype(np.float64)
    deg_d_raw = np.bincount(dst, minlength=n_dst)
    deg_d = np.maximum(deg_d_raw, 1).astype(np.float64)
    rs_s = (1.0 / np.sqrt(deg_s)).astype(np.float32)
    rs_d = (1.0 / np.sqrt(deg_d)).astype(np.float32)
    u_d = np.sqrt(deg_d).astype(np.float32)  # ~= 1/rs_d

    feat_s = (np.asarray(feat, np.float32) * rs_s[:, None]).astype(np.float32)

    D = _rup(_cdiv(n_dst, NCORES), 128)  # dst rows per core (padded)
    ntiles = D // 128

    dst_ids = _balanced_assign(deg_d_raw.astype(np.int64), n_dst, ntiles, rho)
    slot_core = np.empty(n_dst, np.int64)
    slot_loc = np.empty(n_dst, np.int64)
    for c in range(NCORES):
        m = dst_ids[c] >= 0
        slot_core[dst_ids[c, m]] = c
        slot_loc[dst_ids[c, m]] = np.nonzero(m)[0]

    e_core = slot_core[dst]
    e_loc = slot_loc[dst]
    cores = []
    for c in range(NCORES):
        m = e_core == c
        pk = _pack_core(src[m], e_loc[m], n_src, ntiles)
        pk["dst_ids"] = dst_ids[c].copy()
        cores.append(pk)

    # rank-match: per core sort its tiles by slot count desc so tile index t
    # has similar (max-over-core) block counts
    for c in range(NCORES):
        pk = cores[c]
        perm = np.argsort(-pk["nslot"], kind="stable")
        pk["tiles"] = [pk["tiles"][t] for t in perm]
        pk["nslot"] = pk["nslot"][perm]
        pk["nB"] = pk["nB"][perm]
        pk["dst_ids"] = pk["dst_ids"].reshape(ntiles, 128)[perm].reshape(-1)

    nslot_all = np.stack([cores[c]["nslot"] for c in range(NCORES)])
    nB_all = np.stack([cores[c]["nB"] for c in range(NCORES)])
    btile = np.maximum(_cdiv(np.max(nslot_all, axis=0), BLK), 1)
    nbtile = np.minimum(_cdiv(np.max(nB_all, axis=0), BLK), btile)
    nrows = max(len(cores[c]["table_rows"]) for c in range(NCORES))

    return dict(cores=cores, btile=btile, nbtile=nbtile, nrows=nrows,
                nblk=int(btile.sum()), ntiles=ntiles, D=D,
                feat_s=feat_s, rs_d=rs_d, u_d=u_d, n_dst=n_dst)


def _build_host_data(inputs):
    rels = [
        # order matters: output rows are [node_out, inst_out, svc_out].
        # rho = measured pair-merge rate per relation on this graph
        _prep_relation(inputs["in_src"], inputs["in_dst"], INST_N, NODE_N,
                       inputs["instance_feat"], rho=0.49),
        _prep_relation(inputs["ni_src"], inputs["ni_dst"], NODE_N, INST_N,
                       inputs["node_feat"], rho=0.30),
        _prep_relation(inputs["sc_src"], inputs["sc_dst"], SVC_N, SVC_N,
                       inputs["svc_feat"], rho=0.41),
    ]
    Ws = [inputs["W_inst"], inputs["W_node"], inputs["W_svc"]]
    bs = [inputs["b_inst"], inputs["b_node"], inputs["b_svc"]]

    # per-relation gather chunk size minimizing relation-tail pad blocks
    # (pad descriptors are charged by the DMA model even when reg-trimmed)
    def _best_chunk(nblk):
        return min(range(15, 25),
                   key=lambda cc: (_rup(nblk, cc) - nblk, abs(cc - CHUNK)))

    chunks = [_best_chunk(r["nblk"]) for r in rels]
    nblk_pads = [_rup(r["nblk"], chunks[i]) for i, r in enumerate(rels)]
    nblk_tot = sum(nblk_pads)
    nidx_tot = nblk_tot * BLK
    ntile_tot = sum(r["ntiles"] for r in rels)

    W_cat = np.concatenate([np.asarray(w, np.float32) for w in Ws],
                           axis=1).astype(np.float16)
    b_row = np.concatenate([np.asarray(b, np.float32) for b in bs]
                           )[None, :].astype(np.float16)
    iota = np.tile(np.arange(128, dtype=np.float32), (128, 1)).astype(np.float16)

    in_maps = []
    for c in range(NCORES):
        gidx = np.full(nidx_tot, -1, np.int64)
        dlA = np.full(nidx_tot, -1.0, np.float32)
        dlB = np.full(nidx_tot, -1.0, np.float32)
        tbls = []
        rel_bases = np.r_[0, np.cumsum(nblk_pads)]
        for ri, r in enumerate(rels):
            off = int(rel_bases[ri])  # block offset in global stream
            pk = r["cores"][c]
            assert r["nrows"] < 32768, "row idx must fit int16"
            # staggered-duplicate pair rows: tbl[i] = [feat(i), feat(i+1)]
            fr = np.zeros((r["nrows"] + 1, HID), np.float16)
            tr = pk["table_rows"]
            fr[:len(tr)] = r["feat_s"][tr].astype(np.float16)
            tbl = np.concatenate([fr[:-1], fr[1:]], axis=1)
            tbls.append(np.ascontiguousarray(tbl))
            for t in range(r["ntiles"]):
                sp, da, db = pk["tiles"][t]
                n = len(sp)
                base = off * BLK
                gidx[base:base + n] = sp
                gidx[base + n: base + int(r["btile"][t]) * BLK] = 0
                dlA[base:base + n] = da
                dlB[base:base + n] = db
                off += int(r["btile"][t])
            # relation-tail pad blocks keep idx -1 (trimmed device-side)

        idx16 = np.ascontiguousarray(
            gidx.astype(np.int16).reshape(-1, 16).T)
        idx_sb = np.tile(idx16, (8, 1))                          # [128, nidx/16]
        dlA_sb = np.ascontiguousarray(dlA.reshape(nblk_tot, BLK).T)
        dlB_sb = np.ascontiguousarray(dlB.reshape(nblk_tot, BLK).T)

        rs_sb = np.zeros((128, ntile_tot), np.float32)
        u_sb = np.zeros((1, ntile_tot * 128), np.float32)
        t0 = 0
        for r in rels:
            ids = r["cores"][c]["dst_ids"]
            val_rs = np.zeros(r["D"], np.float32)
            val_u = np.zeros(r["D"], np.float32)
            m = ids >= 0
            val_rs[m] = r["rs_d"][ids[m]]
            val_u[m] = r["u_d"][ids[m]]
            rs_sb[:, t0:t0 + r["ntiles"]] = val_rs.reshape(r["ntiles"], 128).T
            u_sb[0, t0 * 128:(t0 + r["ntiles"]) * 128] = val_u
            t0 += r["ntiles"]
        u_sb = u_sb.astype(np.float16)

        in_maps.append({
            "tbl_in": tbls[0],
            "tbl_ni": tbls[1],
            "tbl_sc": tbls[2],
            "idx_sb": np.ascontiguousarray(idx_sb),
            "dlA_sb": dlA_sb,
            "dlB_sb": dlB_sb,
            "rs_sb": rs_sb,
            "u_sb": u_sb,
            "W_cat": np.ascontiguousarray(W_cat),
            "b_row": np.ascontiguousarray(b_row),
            "iota": np.ascontiguousarray(iota),
        })

    meta = dict(
        chunks=chunks,
        has_bias=bool(np.any(b_row != 0)),
        nblk_tot=nblk_tot, nidx_tot=nidx_tot, ntile_tot=ntile_tot,
        nrowss=[r["nrows"] for r in rels],
        btiles=[r["btile"].tolist() for r in rels],
        nbtiles=[r["nbtile"].tolist() for r in rels],
        ntiles=[r["ntiles"] for r in rels],
        Ds=[r["D"] for r in rels],
        n_dsts=[r["n_dst"] for r in rels],
        dst_ids=[[r["cores"][c]["dst_ids"] for c in range(NCORES)]
                 for r in rels],
    )
    return meta, in_maps


def _build_program(meta):
    import os

    import concourse.bacc as bacc
    import concourse.mybir as mybir
    import concourse.tile as tile

    dbg_max_tiles = int(os.environ.get("GNN_MAX_TILES", "0"))  # 0 = all

    gdt = mybir.dt.float16
    f16 = mybir.dt.float16
    f32 = mybir.dt.float32
    AF = mybir.ActivationFunctionType
    act_fn = AF.Lrelu if ACT_MODE == "lrelu" else AF.Relu

    nblk_tot, nidx_tot, ntile_tot = (meta["nblk_tot"], meta["nidx_tot"],
                                     meta["ntile_tot"])

    nc = bacc.Bacc("TRN2", target_bir_lowering=False, debug=False,
                   enable_asserts=False, num_devices=NCORES)

    tbl_d = [
        nc.dram_tensor("tbl_in", [meta["nrowss"][0], PAIR], gdt,
                       kind="ExternalInput"),
        nc.dram_tensor("tbl_ni", [meta["nrowss"][1], PAIR], gdt,
                       kind="ExternalInput"),
        nc.dram_tensor("tbl_sc", [meta["nrowss"][2], PAIR], gdt,
                       kind="ExternalInput"),
    ]
    idx_d = nc.dram_tensor("idx_sb", [128, nidx_tot // 16], mybir.dt.int16,
                           kind="ExternalInput")
    dlA_d = nc.dram_tensor("dlA_sb", [128, nblk_tot], f32, kind="ExternalInput")
    dlB_d = nc.dram_tensor("dlB_sb", [128, nblk_tot], f32, kind="ExternalInput")
    rs_d = nc.dram_tensor("rs_sb", [128, ntile_tot], f32, kind="ExternalInput")
    u_d = nc.dram_tensor("u_sb", [1, ntile_tot * 128], f16, kind="ExternalInput")
    W_d = nc.dram_tensor("W_cat", [128, 3 * HID], f16, kind="ExternalInput")
    b_d = nc.dram_tensor("b_row", [1, 3 * HID], f16, kind="ExternalInput")
    iota_d = nc.dram_tensor("iota", [128, 128], gdt, kind="ExternalInput")

    # p-major grouped layout: row g*128+p holds OUT_GRP tiles' rows for dst
    # slot p — out DMA descriptors are OUT_GRP*256B contiguous (no sub-512B
    # DMA bus penalty); host assemble untangles
    out_d = [
        nc.dram_tensor(n, [_cdiv(meta["ntiles"][i], OUT_GRP) * 128,
                           OUT_GRP * HID], f16, kind="ExternalOutput")
        for i, n in enumerate(["out_node", "out_inst", "out_svc"])
    ]

    with tile.TileContext(nc) as tc:
        with (
            tc.tile_pool(name="const", bufs=1) as const,
            tc.tile_pool(name="g", bufs=int(os.environ.get("GNN_GBUFS", "7"))) as gpool,
            tc.tile_pool(name="st", bufs=int(os.environ.get("GNN_STBUFS", "48"))) as stpool,
            tc.tile_pool(name="evac", bufs=int(os.environ.get("GNN_EVBUFS", "6"))) as evac,
            tc.tile_pool(name="osb", bufs=int(os.environ.get("GNN_OBUFS", "6"))) as opool,
            tc.tile_pool(name="psA", bufs=6, space="PSUM") as psA,
            tc.tile_pool(name="psO", bufs=2, space="PSUM") as psO,
        ):
            rel_nblks = [sum(meta["btiles"][r]) for r in range(3)]
            rel_base = [0, 0, 0]  # global block base per relation
            chs = meta["chunks"]
            for r in range(1, 3):
                rel_base[r] = rel_base[r - 1] + _rup(rel_nblks[r - 1],
                                                     chs[r - 1])
            tg_base = [0, meta["ntiles"][0],
                       meta["ntiles"][0] + meta["ntiles"][1]]

            # first gathers depend only on the leading idx slices + dl/iota:
            # load those first so the gather stream starts ASAP (HWDGE is
            # FIFO per issuing engine)
            idx_t = const.tile([128, nidx_tot // 16], mybir.dt.int16)
            heads = []
            for r in range(3):
                h0 = rel_base[r] * BLK // 16
                h1 = min(h0 + 2 * chs[r] * BLK // 16, nidx_tot // 16)
                heads.append((h0, h1))
                nc.sync.dma_start(idx_t[:, h0:h1], idx_d.ap()[:, h0:h1])
            dlA_t = const.tile([128, nblk_tot], f32)
            nc.sync.dma_start(dlA_t[:], dlA_d.ap())
            dlB_t = const.tile([128, nblk_tot], f32)
            nc.sync.dma_start(dlB_t[:], dlB_d.ap())
            iota_t = const.tile([128, 128], gdt)
            nc.sync.dma_start(iota_t[:], iota_d.ap())
            W_t = const.tile([128, 3 * HID], f16)
            nc.sync.dma_start(W_t[:], W_d.ap())
            b_t = const.tile([1, 3 * HID], f16)
            nc.sync.dma_start(b_t[:], b_d.ap())
            u_t = const.tile([1, ntile_tot * 128], f16)
            nc.sync.dma_start(u_t[:], u_d.ap())
            rs_t = const.tile([128, ntile_tot], f32)
            nc.sync.dma_start(rs_t[:], rs_d.ap())
            for r in range(3):
                h1 = heads[r][1]
                end = rel_base[r] * BLK // 16 + \
                    _rup(rel_nblks[r], chs[r]) * BLK // 16
                if h1 < end:
                    nc.sync.dma_start(idx_t[:, h1:end], idx_d.ap()[:, h1:end])

            g_tiles = {}   # (rel, rel-local chunk) -> gather tile

            def issue_gather(rel, lci):
                ch = chs[rel]
                # last chunk: shrink to the real remainder so the cost-model
                # (and HW) never touches relation-tail pad descriptors
                real_blocks = max(1, min(ch, rel_nblks[rel] - lci * ch))
                gt = gpool.tile([128, real_blocks, PAIR], gdt, tag="g",
                                name="gt")
                nidx = real_blocks * BLK
                h0 = rel_base[rel] * BLK // 16 + lci * (ch * BLK // 16)
                nc.gpsimd.dma_gather(
                    out_ap=gt[:],
                    in_ap=tbl_d[rel].ap(),
                    idxs_ap=idx_t[:, h0:h0 + nidx // 16],
                    num_idxs=nidx,
                    num_idxs_reg=nidx,
                    elem_size=PAIR,
                    single_packet=False,
                )
                g_tiles[(rel, lci)] = gt

            # relation-interleaved tile schedule: spread the epilogue-heavy
            # relation (ni: many tiles, few blocks) evenly across the
            # gather-heavy one (sc) so no engine's work bunches up
            sched = []
            scale = [0.93, 0.95, 1.0]  # in/ni finish early; sc's last blocks
            for r in range(3):         # keep gathers flowing during drain
                btile = meta["btiles"][r]
                o = 0
                for t in range(meta["ntiles"][r]):
                    if dbg_max_tiles and t >= dbg_max_tiles:
                        break
                    # key on end-fraction: the block-heavy relation's last
                    # tile keeps gathers flowing while light tiles' epilogues
                    # drain, shrinking the no-DMA tail
                    sched.append((scale[r] * (o + btile[t])
                                  / max(1, rel_nblks[r]), r, t))
                    o += btile[t]
            sched.sort()

            rel_blk = [0, 0, 0]       # relation-local block cursor
            osb_state = [None, None, None]

            for _, rel, t in sched:
                ntiles = meta["ntiles"][rel]
                nb = meta["btiles"][rel][t]
                nbB = meta["nbtiles"][rel][t]
                agg = psA.tile([128, 128], f32, tag="agg")
                for b in range(nb):
                    lb = rel_blk[rel]
                    lci, cj = divmod(lb, chs[rel])
                    if cj == 0:
                        issue_gather(rel, lci)
                    blk = rel_base[rel] + lb  # global dl column
                    has_b = b < nbB
                    stA = stpool.tile([128, 128], gdt, tag="stA")
                    nc.vector.tensor_scalar(
                        stA[:], iota_t[:], dlA_t[:, blk:blk + 1], None,
                        mybir.AluOpType.is_equal)
                    last = b == nb - 1
                    nc.tensor.matmul(
                        agg[:], g_tiles[(rel, lci)][:, cj, 0:HID], stA[:],
                        start=(b == 0), stop=(last and not has_b))
                    if has_b:
                        stB = stpool.tile([128, 128], gdt, tag="stB")
                        nc.vector.tensor_scalar(
                            stB[:], iota_t[:], dlB_t[:, blk:blk + 1], None,
                            mybir.AluOpType.is_equal)
                        nc.tensor.matmul(
                            agg[:], g_tiles[(rel, lci)][:, cj, HID:PAIR], stB[:],
                            start=False, stop=last)
                    rel_blk[rel] += 1
                tglob = tg_base[rel] + t
                aggsb = evac.tile([128, 128], f16, tag="evac")
                nc.scalar.copy(aggsb[:], agg[:])
                po = psO.tile([128, 128], f32, tag="po")
                if meta["has_bias"]:
                    nc.tensor.matmul(
                        po[:], u_t[:, tglob * 128:(tglob + 1) * 128],
                        b_t[:, rel * HID:(rel + 1) * HID],
                        start=True, stop=False, skip_group_check=True)
                nc.tensor.matmul(
                    po[:], aggsb[:], W_t[:, rel * HID:(rel + 1) * HID],
                    start=not meta["has_bias"], stop=True,
                    skip_group_check=True)
                oj = t % OUT_GRP
                if oj == 0:
                    osb_state[rel] = (
                        opool.tile([128, OUT_GRP, 128], f16, tag="osb",
                                   name="osb"), t)
                osb, osb_t0 = osb_state[rel]
                nc.scalar.activation(
                    osb[:, oj, :], po[:], act_fn,
                    bias=0.0, scale=rs_t[:, tglob:tglob + 1], alpha=0.01)
                if oj == OUT_GRP - 1 or t == ntiles - 1:
                    cnt = t - osb_t0 + 1
                    g = osb_t0 // OUT_GRP
                    dst = out_d[rel].ap()[g * 128:(g + 1) * 128, :cnt * HID]
                    nc.sync.dma_start(
                        dst.rearrange("p (j k) -> p j k", k=HID),
                        osb[:, :cnt, :])

    nc.compile()
    return nc


def _run(nc, in_maps, trace=False, **kw):
    from concourse import bass_utils
    res = bass_utils.run_bass_kernel_spmd(
        nc, in_maps, core_ids=list(range(NCORES)), trace=trace, **kw)
    return res


def _assemble(results, meta):
    out = np.empty((NODE_N + INST_N + SVC_N, HID), np.float32)
    offs = [0, NODE_N, NODE_N + INST_N]
    names = ["out_node", "out_inst", "out_svc"]
    for rel in range(3):
        nt = meta["ntiles"][rel]
        ngrp = _cdiv(nt, OUT_GRP)
        for c in range(NCORES):
            ids = meta["dst_ids"][rel][c]
            m = ids >= 0
            arr = np.asarray(results[c][names[rel]], np.float32)
            rows = arr.reshape(ngrp, 128, OUT_GRP, HID).transpose(
                0, 2, 1, 3).reshape(ngrp * OUT_GRP * 128, HID)[:nt * 128]
            out[offs[rel] + ids[m]] = rows[m]
    return out


def kernel(**inputs):
    import hashlib
    key = "prog"
    h = hashlib.sha1()
    for k in ("sc_src", "sc_dst", "in_src", "in_dst", "ni_src", "ni_dst"):
        h.update(np.ascontiguousarray(np.asarray(inputs[k], np.int32)).tobytes())
    sig = h.hexdigest()
    meta, in_maps = _build_host_data(inputs)
    if key in _cache and _cache[key][0] == sig:
        _, nc, _ = _cache[key]
    else:
        nc = _build_program(meta)
        _cache[key] = (sig, nc, meta)
    res = _run(nc, in_maps)
    return _assemble(res.results, meta)


# revision 66
# speedup vs baseline: 1.0757x; 1.0052x over previous
"""Trainium2 Bass kernel for a heterogeneous GraphConv layer (3 relations).

out = concat([leaky(GC(inst_feat, W_inst, in_*)),     # -> node   (10000)
              leaky(GC(node_feat, W_node, ni_*)),     # -> inst   (100000)
              leaky(GC(svc_feat,  W_svc,  sc_*))])    # -> svc    (20000)

GC(f, W, src, dst) = rsqrt(deg_d) * segsum_dst((rsqrt(deg_s)*f)[src]) @ W + b
(aggregation commutes with the dense @W, so we gather *raw scaled features*
and apply W once per destination tile).

Strategy: destination-sharded across 8 NeuronCores, with host-balanced
dst->(core,tile,slot) assignment (tile edge sums packed to multiples of 128
and rank-matched across cores so the shared program's per-tile block counts
stay near the mean).

DMA-gather cost on TRN2 is per-descriptor: a 512B descriptor costs the same
as 256B (sub-512B transfers pay a 2x bus penalty), so the gather tables are
laid out as PAIR rows [2*128] fp16 = 512B: one descriptor fetches two
feature rows. The per-core table is ordered by first use so the two edges
that introduce a pair of new sources in the same dst tile share one
descriptor ("paired slots"); repeat edges use one half and the other half
rides free (and occasionally serves a second edge whose source lands on the
pair sibling). Per slot there are two one-hot channels dlA/dlB (-1 = unused)
selecting the dst column for the low/high half row.

Device work per 128-slot block: dma_gather 128 pair rows -> [128e, 256f],
DVE tensor_scalar builds one-hot stA[e,d]=(iota==dlA) (and stB for the
pair-slot prefix blocks only), PE accumulates aggT[f,d] += A.T@stA (+B.T@stB)
in PSUM. Per dst tile: PSUM out = aggT.T @ W (fp16; plus a rank-1 u (x) b
bias-preload matmul only when the bias row is nonzero), then one ScalarE
Lrelu(out * rsqrt_deg_d) and a grouped p-major fp16 DMA (OUT_GRP tiles,
>=512B descriptors) to the output rows. Tiles of the three relations are
interleaved by end-fraction so epilogue-heavy tiles spread across the
gather-bound stream.
"""

import os as _os

import numpy as np

SVC_N, INST_N, NODE_N, HID = 20000, 100000, 10000, 128
NCORES = 8
BLK = 128           # slots per one-hot matmul block
# blocks per dma_gather instruction. NOTE: needs single_packet=False — with
# single_packet=True the whole stream coalesces into one DMA packet, which
# caps at 64 descriptors/engine (num_idxs <= 1024); beyond that the exec
# unit faults (NRT_EXEC_UNIT_UNRECOVERABLE).
CHUNK = int(_os.environ.get("GNN_CHUNK", "16"))
GDT = "fp16"
ACT_MODE = "lrelu"  # "lrelu" (HW leaky relu) | "relu" (sim debug)
PAIR = 2 * HID      # table row = pair of feature rows (512B fp16)
# dst tiles batched per epilogue staging buffer / out DMA
OUT_GRP = int(_os.environ.get("GNN_OUT_GRP", "16"))

_cache = {}


def _cdiv(a, b):
    return (a + b - 1) // b


def _rup(a, b):
    return _cdiv(a, b) * b


def _balanced_assign(deg, n_dst, ntiles, rho):
    """Assign dst nodes to (core, tile, slot) packing per-tile SLOT counts
    (estimated as edges*(1-rho), rho = relation merge rate) just under
    multiples of 128 so per-tile block counts carry minimal ceil padding.

    Returns dst_ids[NCORES, ntiles*128] int64 (-1 = pad slot): the global dst
    node stored at each (core, tile, slot).
    """
    order = np.argsort(-deg, kind="stable")
    # snake over cores -> per-core totals equal to within one max-degree
    core_of = np.empty(n_dst, np.int64)
    snake = np.concatenate([np.arange(NCORES), np.arange(NCORES)[::-1]])
    core_of[order] = snake[np.arange(n_dst) % (2 * NCORES)]

    dst_ids = np.full((NCORES, ntiles * 128), -1, np.int64)
    for c in range(NCORES):
        ids = order[core_of[order] == c]  # degree-descending
        w = deg[ids]
        total = int(w.sum())
        # uniform edge-weight target per bin (multiple of 128; rho reserved
        # for a future slot-aware quota scheme — measured merge-rate noise
        # and the max-over-core coupling made per-bin quotas regress)
        target = np.full(ntiles, _cdiv(total, ntiles * 128) * 128, np.int64)
        binw = np.zeros(ntiles, np.int64)
        binn = np.zeros(ntiles, np.int64)
        bins = [[] for _ in range(ntiles)]
        # greedy: place each dst (deg desc) in the fullest bin it still fits
        # (by weight target and 128-slot cap); else least-filled open bin
        open_bins = list(range(ntiles))
        for i, d in zip(ids, w):
            best, bestw = -1, -1
            for t in open_bins:
                if binw[t] + d <= target[t] and binw[t] > bestw:
                    best, bestw = t, binw[t]
            if best < 0:
                best = min(open_bins, key=lambda x: binw[x])
            t = best
            bins[t].append(i)
            binw[t] += d
            binn[t] += 1
            if binn[t] >= 128:
                open_bins.remove(t)
        for t in range(ntiles):
            ids_t = bins[t]
            dst_ids[c, t * 128: t * 128 + len(ids_t)] = ids_t
    return dst_ids


def _pack_core(es, ed, n_src, ntiles):
    """Pair-slot packing for one (core, relation).

    es: edge source node ids; ed: edge dst slot (tile*128 + dst_local).

    The gather table holds one 512B row per USED source: [feat(r), feat(r+1)]
    (staggered duplicate), so descriptor idx r serves edge(s) on table row r
    via channel A and optionally a second edge on row r+1 via channel B.
    Rows are ordered by their tile-usage lists (lexsort) so edges of the same
    tile sit on adjacent rows and merge into shared slots.

    Returns dict with: table_rows (src id per table row), per-tile slot
    arrays (row idx, dlA, dlB), nslot[t], nB[t].
    """
    KEYLEN = 6
    tile_of = (ed >> 7).astype(np.int64)
    dl = (ed & 127).astype(np.int64)

    rows_used = np.unique(es)
    nrows = len(rows_used)
    rid_of = np.full(n_src, -1, np.int64)
    rid_of[rows_used] = np.arange(nrows)

    # tile-usage key per row: first KEYLEN tiles (sorted), padded
    pt = np.unique(np.stack([rid_of[es], tile_of], axis=1), axis=0)
    grp_new = np.r_[True, pt[1:, 0] != pt[:-1, 0]]
    idx = np.arange(len(pt))
    j = idx - np.maximum.accumulate(np.where(grp_new, idx, 0))
    keymat = np.full((nrows, KEYLEN), 32767, np.int64)
    m = j < KEYLEN
    keymat[pt[m, 0], j[m]] = pt[m, 1]
    order = np.lexsort(keymat.T[::-1])
    table_rows = rows_used[order]          # src id at each table position
    pos_of = np.full(n_src, -1, np.int64)
    pos_of[table_rows] = np.arange(nrows)

    r = pos_of[es]
    # per tile: sort edges by table position; pair edges on consecutive
    # positions (runs split on gaps/duplicates, paired (0,1)(2,3)... in-run)
    o = np.lexsort((r, tile_of))
    kt, kr, kdl = tile_of[o], r[o], dl[o]
    brk = np.r_[True, (kt[1:] != kt[:-1]) | (kr[1:] != kr[:-1] + 1)]
    idx = np.arange(len(kt))
    k_in_run = idx - np.maximum.accumulate(np.where(brk, idx, 0))
    run_id = np.cumsum(brk) - 1
    half = k_in_run & 1
    skey = np.stack([run_id, k_in_run >> 1], axis=1)
    uslot, inv = np.unique(skey, axis=0, return_inverse=True)
    ns = len(uslot)
    s_tile = np.zeros(ns, np.int64)
    s_row = np.zeros(ns, np.int64)
    dlA = np.full(ns, -1, np.int64)
    dlB = np.full(ns, -1, np.int64)
    m0 = half == 0
    s_tile[inv[m0]] = kt[m0]
    s_row[inv[m0]] = kr[m0]
    dlA[inv[m0]] = kdl[m0]
    dlB[inv[~m0]] = kdl[~m0]
    has_b = dlB >= 0

    # order slots per tile: B-present first (so stB/matmul-B run only on a
    # prefix of blocks), then by row for gather locality
    so = np.lexsort((s_row, ~has_b, s_tile))
    s_tile, s_row, dlA, dlB, has_b = (
        s_tile[so], s_row[so], dlA[so], dlB[so], has_b[so])

    nslot = np.bincount(s_tile, minlength=ntiles)
    nB = np.bincount(s_tile[has_b], minlength=ntiles)
    tstart = np.r_[0, np.cumsum(nslot)]
    tiles = []
    for t in range(ntiles):
        sl = slice(tstart[t], tstart[t + 1])
        tiles.append((s_row[sl], dlA[sl], dlB[sl]))
    return dict(table_rows=table_rows, tiles=tiles, nslot=nslot, nB=nB)


def _prep_relation(src, dst, n_src, n_dst, feat, rho):
    """Host-side sharding/packing for one relation.

    rho: estimated slot merge rate (pair-served edge fraction) used to pack
    tiles to near-multiple-of-128 slot counts.
    """
    src = np.asarray(src, np.int64)
    dst = np.asarray(dst, np.int64)
    deg_s = np.maximum(np.bincount(src, minlength=n_src), 1).astype(np.float64)
    deg_d_raw = np.bincount(dst, minlength=n_dst)
    deg_d = np.maximum(deg_d_raw, 1).astype(np.float64)
    rs_s = (1.0 / np.sqrt(deg_s)).astype(np.float32)
    rs_d = (1.0 / np.sqrt(deg_d)).astype(np.float32)
    u_d = np.sqrt(deg_d).astype(np.float32)  # ~= 1/rs_d

    feat_s = (np.asarray(feat, np.float32) * rs_s[:, None]).astype(np.float32)

    D = _rup(_cdiv(n_dst, NCORES), 128)  # dst rows per core (padded)
    ntiles = D // 128

    dst_ids = _balanced_assign(deg_d_raw.astype(np.int64), n_dst, ntiles, rho)
    slot_core = np.empty(n_dst, np.int64)
    slot_loc = np.empty(n_dst, np.int64)
    for c in range(NCORES):
        m = dst_ids[c] >= 0
        slot_core[dst_ids[c, m]] = c
        slot_loc[dst_ids[c, m]] = np.nonzero(m)[0]

    e_core = slot_core[dst]
    e_loc = slot_loc[dst]
    cores = []
    for c in range(NCORES):
        m = e_core == c
        pk = _pack_core(src[m], e_loc[m], n_src, ntiles)
        pk["dst_ids"] = dst_ids[c].copy()
        cores.append(pk)

    # rank-match: per core sort its tiles by slot count desc so tile index t
    # has similar (max-over-core) block counts
    for c in range(NCORES):
        pk = cores[c]
        perm = np.argsort(-pk["nslot"], kind="stable")
        pk["tiles"] = [pk["tiles"][t] for t in perm]
        pk["nslot"] = pk["nslot"][perm]
        pk["nB"] = pk["nB"][perm]
        pk["dst_ids"] = pk["dst_ids"].reshape(ntiles, 128)[perm].reshape(-1)

    nslot_all = np.stack([cores[c]["nslot"] for c in range(NCORES)])
    nB_all = np.stack([cores[c]["nB"] for c in range(NCORES)])
    btile = np.maximum(_cdiv(np.max(nslot_all, axis=0), BLK), 1)
    nbtile = np.minimum(_cdiv(np.max(nB_all, axis=0), BLK), btile)
    nrows = max(len(cores[c]["table_rows"]) for c in range(NCORES))

    return dict(cores=cores, btile=btile, nbtile=nbtile, nrows=nrows,
                nblk=int(btile.sum()), ntiles=ntiles, D=D,
                feat_s=feat_s, rs_d=rs_d, u_d=u_d, n_dst=n_dst)


def _build_host_data(inputs):
    rels = [
        # order matters: output rows are [node_out, inst_out, svc_out].
        # rho = measured pair-merge rate per relation on this graph
        _prep_relation(inputs["in_src"], inputs["in_dst"], INST_N, NODE_N,
                       inputs["instance_feat"], rho=0.49),
        _prep_relation(inputs["ni_src"], inputs["ni_dst"], NODE_N, INST_N,
                       inputs["node_feat"], rho=0.30),
        _prep_relation(inputs["sc_src"], inputs["sc_dst"], SVC_N, SVC_N,
                       inputs["svc_feat"], rho=0.41),
    ]
    Ws = [inputs["W_inst"], inputs["W_node"], inputs["W_svc"]]
    bs = [inputs["b_inst"], inputs["b_node"], inputs["b_svc"]]

    # per-relation gather chunk size minimizing relation-tail pad blocks
    # (pad descriptors are charged by the DMA model even when reg-trimmed)
    def _best_chunk(nblk):
        return min(range(15, 25),
                   key=lambda cc: (_rup(nblk, cc) - nblk, abs(cc - CHUNK)))

    chunks = [_best_chunk(r["nblk"]) for r in rels]
    nblk_pads = [_rup(r["nblk"], chunks[i]) for i, r in enumerate(rels)]
    nblk_tot = sum(nblk_pads)
    nidx_tot = nblk_tot * BLK
    ntile_tot = sum(r["ntiles"] for r in rels)

    W_cat = np.concatenate([np.asarray(w, np.float32) for w in Ws],
                           axis=1).astype(np.float16)
    b_row = np.concatenate([np.asarray(b, np.float32) for b in bs]
                           )[None, :].astype(np.float16)
    iota = np.tile(np.arange(128, dtype=np.float32), (128, 1)).astype(np.float16)

    in_maps = []
    for c in range(NCORES):
        gidx = np.full(nidx_tot, -1, np.int64)
        dlA = np.full(nidx_tot, -1.0, np.float32)
        dlB = np.full(nidx_tot, -1.0, np.float32)
        tbls = []
        rel_bases = np.r_[0, np.cumsum(nblk_pads)]
        for ri, r in enumerate(rels):
            off = int(rel_bases[ri])  # block offset in global stream
            pk = r["cores"][c]
            assert r["nrows"] < 32768, "row idx must fit int16"
            # staggered-duplicate pair rows: tbl[i] = [feat(i), feat(i+1)]
            fr = np.zeros((r["nrows"] + 1, HID), np.float16)
            tr = pk["table_rows"]
            fr[:len(tr)] = r["feat_s"][tr].astype(np.float16)
            tbl = np.concatenate([fr[:-1], fr[1:]], axis=1)
            tbls.append(np.ascontiguousarray(tbl))
            for t in range(r["ntiles"]):
                sp, da, db = pk["tiles"][t]
                n = len(sp)
                base = off * BLK
                gidx[base:base + n] = sp
                gidx[base + n: base + int(r["btile"][t]) * BLK] = 0
                dlA[base:base + n] = da
                dlB[base:base + n] = db
                off += int(r["btile"][t])
            # relation-tail pad blocks keep idx -1 (trimmed device-side)

        idx16 = np.ascontiguousarray(
            gidx.astype(np.int16).reshape(-1, 16).T)
        idx_sb = np.tile(idx16, (8, 1))                          # [128, nidx/16]
        dlA_sb = np.ascontiguousarray(dlA.reshape(nblk_tot, BLK).T)
        dlB_sb = np.ascontiguousarray(dlB.reshape(nblk_tot, BLK).T)

        rs_sb = np.zeros((128, ntile_tot), np.float32)
        u_sb = np.zeros((1, ntile_tot * 128), np.float32)
        t0 = 0
        for r in rels:
            ids = r["cores"][c]["dst_ids"]
            val_rs = np.zeros(r["D"], np.float32)
            val_u = np.zeros(r["D"], np.float32)
            m = ids >= 0
            val_rs[m] = r["rs_d"][ids[m]]
            val_u[m] = r["u_d"][ids[m]]
            rs_sb[:, t0:t0 + r["ntiles"]] = val_rs.reshape(r["ntiles"], 128).T
            u_sb[0, t0 * 128:(t0 + r["ntiles"]) * 128] = val_u
            t0 += r["ntiles"]
        u_sb = u_sb.astype(np.float16)

        in_maps.append({
            "tbl_in": tbls[0],
            "tbl_ni": tbls[1],
            "tbl_sc": tbls[2],
            "idx_sb": np.ascontiguousarray(idx_sb),
            "dlA_sb": dlA_sb,
            "dlB_sb": dlB_sb,
            "rs_sb": rs_sb,
            "u_sb": u_sb,
            "W_cat": np.ascontiguousarray(W_cat),
            "b_row": np.ascontiguousarray(b_row),
            "iota": np.ascontiguousarray(iota),
        })

    meta = dict(
        chunks=chunks,
        has_bias=bool(np.any(b_row != 0)),
        nblk_tot=nblk_tot, nidx_tot=nidx_tot, ntile_tot=ntile_tot,
        nrowss=[r["nrows"] for r in rels],
        btiles=[r["btile"].tolist() for r in rels],
        nbtiles=[r["nbtile"].tolist() for r in rels],
        ntiles=[r["ntiles"] for r in rels],
        Ds=[r["D"] for r in rels],
        n_dsts=[r["n_dst"] for r in rels],
        dst_ids=[[r["cores"][c]["dst_ids"] for c in range(NCORES)]
                 for r in rels],
    )
    return meta, in_maps


def _build_program(meta):
    import os

    import concourse.bacc as bacc
    import concourse.mybir as mybir
    import concourse.tile as tile

    dbg_max_tiles = int(os.environ.get("GNN_MAX_TILES", "0"))  # 0 = all

    gdt = mybir.dt.float16
    f16 = mybir.dt.float16
    f32 = mybir.dt.float32
    AF = mybir.ActivationFunctionType
    act_fn = AF.Lrelu if ACT_MODE == "lrelu" else AF.Relu

    nblk_tot, nidx_tot, ntile_tot = (meta["nblk_tot"], meta["nidx_tot"],
                                     meta["ntile_tot"])

    nc = bacc.Bacc("TRN2", target_bir_lowering=False, debug=False,
                   enable_asserts=False, num_devices=NCORES)

    tbl_d = [
        nc.dram_tensor("tbl_in", [meta["nrowss"][0], PAIR], gdt,
                       kind="ExternalInput"),
        nc.dram_tensor("tbl_ni", [meta["nrowss"][1], PAIR], gdt,
                       kind="ExternalInput"),
        nc.dram_tensor("tbl_sc", [meta["nrowss"][2], PAIR], gdt,
                       kind="ExternalInput"),
    ]
    idx_d = nc.dram_tensor("idx_sb", [128, nidx_tot // 16], mybir.dt.int16,
                           kind="ExternalInput")
    dlA_d = nc.dram_tensor("dlA_sb", [128, nblk_tot], f32, kind="ExternalInput")
    dlB_d = nc.dram_tensor("dlB_sb", [128, nblk_tot], f32, kind="ExternalInput")
    rs_d = nc.dram_tensor("rs_sb", [128, ntile_tot], f32, kind="ExternalInput")
    u_d = nc.dram_tensor("u_sb", [1, ntile_tot * 128], f16, kind="ExternalInput")
    W_d = nc.dram_tensor("W_cat", [128, 3 * HID], f16, kind="ExternalInput")
    b_d = nc.dram_tensor("b_row", [1, 3 * HID], f16, kind="ExternalInput")
    iota_d = nc.dram_tensor("iota", [128, 128], gdt, kind="ExternalInput")

    # p-major grouped layout: row g*128+p holds OUT_GRP tiles' rows for dst
    # slot p — out DMA descriptors are OUT_GRP*256B contiguous (no sub-512B
    # DMA bus penalty); host assemble untangles
    out_d = [
        nc.dram_tensor(n, [_cdiv(meta["ntiles"][i], OUT_GRP) * 128,
                           OUT_GRP * HID], f16, kind="ExternalOutput")
        for i, n in enumerate(["out_node", "out_inst", "out_svc"])
    ]

    with tile.TileContext(nc) as tc:
        with (
            tc.tile_pool(name="const", bufs=1) as const,
            tc.tile_pool(name="g", bufs=int(os.environ.get("GNN_GBUFS", "7"))) as gpool,
            tc.tile_pool(name="st", bufs=int(os.environ.get("GNN_STBUFS", "48"))) as stpool,
            tc.tile_pool(name="evac", bufs=int(os.environ.get("GNN_EVBUFS", "6"))) as evac,
            tc.tile_pool(name="osb", bufs=int(os.environ.get("GNN_OBUFS", "6"))) as opool,
            tc.tile_pool(name="psA", bufs=6, space="PSUM") as psA,
            tc.tile_pool(name="psO", bufs=2, space="PSUM") as psO,
        ):
            rel_nblks = [sum(meta["btiles"][r]) for r in range(3)]
            rel_base = [0, 0, 0]  # global block base per relation
            chs = meta["chunks"]
            for r in range(1, 3):
                rel_base[r] = rel_base[r - 1] + _rup(rel_nblks[r - 1],
                                                     chs[r - 1])
            tg_base = [0, meta["ntiles"][0],
                       meta["ntiles"][0] + meta["ntiles"][1]]

            # first gathers depend only on the leading idx slices + dl/iota:
            # load those first so the gather stream starts ASAP (HWDGE is
            # FIFO per issuing engine)
            idx_t = const.tile([128, nidx_tot // 16], mybir.dt.int16)
            heads = []
            for r in range(3):
                h0 = rel_base[r] * BLK // 16
                h1 = min(h0 + 6 * chs[r] * BLK // 16, nidx_tot // 16)
                heads.append((h0, h1))
                nc.sync.dma_start(idx_t[:, h0:h1], idx_d.ap()[:, h0:h1])
            dlA_t = const.tile([128, nblk_tot], f32)
            nc.sync.dma_start(dlA_t[:], dlA_d.ap())
            dlB_t = const.tile([128, nblk_tot], f32)
            nc.sync.dma_start(dlB_t[:], dlB_d.ap())
            iota_t = const.tile([128, 128], gdt)
            nc.sync.dma_start(iota_t[:], iota_d.ap())
            W_t = const.tile([128, 3 * HID], f16)
            nc.sync.dma_start(W_t[:], W_d.ap())
            b_t = const.tile([1, 3 * HID], f16)
            nc.sync.dma_start(b_t[:], b_d.ap())
            u_t = const.tile([1, ntile_tot * 128], f16)
            nc.sync.dma_start(u_t[:], u_d.ap())
            rs_t = const.tile([128, ntile_tot], f32)
            nc.sync.dma_start(rs_t[:], rs_d.ap())
            for r in range(3):
                h1 = heads[r][1]
                end = rel_base[r] * BLK // 16 + \
                    _rup(rel_nblks[r], chs[r]) * BLK // 16
                if h1 < end:
                    nc.sync.dma_start(idx_t[:, h1:end], idx_d.ap()[:, h1:end])

            g_tiles = {}   # (rel, rel-local chunk) -> gather tile

            def issue_gather(rel, lci):
                ch = chs[rel]
                # last chunk: shrink to the real remainder so the cost-model
                # (and HW) never touches relation-tail pad descriptors
                real_blocks = max(1, min(ch, rel_nblks[rel] - lci * ch))
                gt = gpool.tile([128, real_blocks, PAIR], gdt, tag="g",
                                name="gt")
                nidx = real_blocks * BLK
                h0 = rel_base[rel] * BLK // 16 + lci * (ch * BLK // 16)
                nc.gpsimd.dma_gather(
                    out_ap=gt[:],
                    in_ap=tbl_d[rel].ap(),
                    idxs_ap=idx_t[:, h0:h0 + nidx // 16],
                    num_idxs=nidx,
                    num_idxs_reg=nidx,
                    elem_size=PAIR,
                    single_packet=False,
                )
                g_tiles[(rel, lci)] = gt

            # relation-interleaved tile schedule: spread the epilogue-heavy
            # relation (ni: many tiles, few blocks) evenly across the
            # gather-heavy one (sc) so no engine's work bunches up
            sched = []
            scale = [0.93, 0.95, 1.0]  # in/ni finish early; sc's last blocks
            for r in range(3):         # keep gathers flowing during drain
                btile = meta["btiles"][r]
                o = 0
                for t in range(meta["ntiles"][r]):
                    if dbg_max_tiles and t >= dbg_max_tiles:
                        break
                    # key on end-fraction: the block-heavy relation's last
                    # tile keeps gathers flowing while light tiles' epilogues
                    # drain, shrinking the no-DMA tail
                    sched.append((scale[r] * (o + btile[t])
                                  / max(1, rel_nblks[r]), r, t))
                    o += btile[t]
            sched.sort()

            rel_blk = [0, 0, 0]       # relation-local block cursor
            osb_state = [None, None, None]

            for _, rel, t in sched:
                ntiles = meta["ntiles"][rel]
                nb = meta["btiles"][rel][t]
                nbB = meta["nbtiles"][rel][t]
                agg = psA.tile([128, 128], f32, tag="agg")
                for b in range(nb):
                    lb = rel_blk[rel]
                    lci, cj = divmod(lb, chs[rel])
                    if cj == 0:
                        issue_gather(rel, lci)
                    blk = rel_base[rel] + lb  # global dl column
                    has_b = b < nbB
                    stA = stpool.tile([128, 128], gdt, tag="stA")
                    nc.vector.tensor_scalar(
                        stA[:], iota_t[:], dlA_t[:, blk:blk + 1], None,
                        mybir.AluOpType.is_equal)
                    last = b == nb - 1
                    nc.tensor.matmul(
                        agg[:], g_tiles[(rel, lci)][:, cj, 0:HID], stA[:],
                        start=(b == 0), stop=(last and not has_b))
                    if has_b:
                        stB = stpool.tile([128, 128], gdt, tag="stB")
                        nc.vector.tensor_scalar(
                            stB[:], iota_t[:], dlB_t[:, blk:blk + 1], None,
                            mybir.AluOpType.is_equal)
                        nc.tensor.matmul(
                            agg[:], g_tiles[(rel, lci)][:, cj, HID:PAIR], stB[:],
                            start=False, stop=last)
                    rel_blk[rel] += 1
                tglob = tg_base[rel] + t
                aggsb = evac.tile([128, 128], f16, tag="evac")
                nc.scalar.copy(aggsb[:], agg[:])
                po = psO.tile([128, 128], f32, tag="po")
                if meta["has_bias"]:
                    nc.tensor.matmul(
                        po[:], u_t[:, tglob * 128:(tglob + 1) * 128],
                        b_t[:, rel * HID:(rel + 1) * HID],
                        start=True, stop=False, skip_group_check=True)
                nc.tensor.matmul(
                    po[:], aggsb[:], W_t[:, rel * HID:(rel + 1) * HID],
                    start=not meta["has_bias"], stop=True,
                    skip_group_check=True)
                oj = t % OUT_GRP
                if oj == 0:
                    osb_state[rel] = (
                        opool.tile([128, OUT_GRP, 128], f16, tag="osb",
                                   name="osb"), t)
                osb, osb_t0 = osb_state[rel]
                nc.scalar.activation(
                    osb[:, oj, :], po[:], act_fn,
                    bias=0.0, scale=rs_t[:, tglob:tglob + 1], alpha=0.01)
                if oj == OUT_GRP - 1 or t == ntiles - 1:
                    cnt = t - osb_t0 + 1
                    g = osb_t0 // OUT_GRP
                    dst = out_d[rel].ap()[g * 128:(g + 1) * 128, :cnt * HID]
                    nc.sync.dma_start(
                        dst.rearrange("p (j k) -> p j k", k=HID),
                        osb[:, :cnt, :])

    nc.compile()
    return nc


def _run(nc, in_maps, trace=False, **kw):
    from concourse import bass_utils
    res = bass_utils.run_bass_kernel_spmd(
        nc, in_maps, core_ids=list(range(NCORES)), trace=trace, **kw)
    return res


def _assemble(results, meta):
    out = np.empty((NODE_N + INST_N + SVC_N, HID), np.float32)
    offs = [0, NODE_N, NODE_N + INST_N]
    names = ["out_node", "out_inst", "out_svc"]
    for rel in range(3):
        nt = meta["ntiles"][rel]
        ngrp = _cdiv(nt, OUT_GRP)
        for c in range(NCORES):
            ids = meta["dst_ids"][rel][c]
            m = ids >= 0
            arr = np.asarray(results[c][names[rel]], np.float32)
            rows = arr.reshape(ngrp, 128, OUT_GRP, HID).transpose(
                0, 2, 1, 3).reshape(ngrp * OUT_GRP * 128, HID)[:nt * 128]
            out[offs[rel] + ids[m]] = rows[m]
    return out


def kernel(**inputs):
    import hashlib
    key = "prog"
    h = hashlib.sha1()
    for k in ("sc_src", "sc_dst", "in_src", "in_dst", "ni_src", "ni_dst"):
        h.update(np.ascontiguousarray(np.asarray(inputs[k], np.int32)).tobytes())
    sig = h.hexdigest()
    meta, in_maps = _build_host_data(inputs)
    if key in _cache and _cache[key][0] == sig:
        _, nc, _ = _cache[key]
    else:
        nc = _build_program(meta)
        _cache[key] = (sig, nc, meta)
    res = _run(nc, in_maps)
    return _assemble(res.results, meta)
